# revision 1
# baseline (speedup 1.0000x reference)
"""Trainium2 Bass kernel for nn_Encoder (B=4, S=2048, D=512, H=8 self-attention).

Sharding over 8 NeuronCores: core c -> (batch b = c//2, head-group hg = c%2).
Each core computes, for its batch and its 4 heads, the full attention block
plus a partial output projection y_part = attn_out @ Wo[group rows]. The host
sums the two partial y tensors per batch.

v2 layout: phases iterate (tw, pp) where tw is a 512-wide query window and pp
a head pair. Per key tile st, both heads' scores land in one [128, 1024] PSUM
tile (one bank per head) so a single Exp activation covers the pair (mask bias
is per-partition = per-key, identical for both heads). attnV accumulates
[65, 512] per head (ones column -> denominator row). PSUM budget: scores
2 tiles x 2 banks + attnV 3 x 1 + scratch 1 x 1 = 8 banks.

Scheduling: the Exp stream on the Scalar engine is the pacing resource, so
everything else is shaped around keeping it saturated: x is DMAd in s-column
pieces so the first projection chains complete right after the first 128 KB
lands, dummy matmuls warm the PE (HAM un-throttle) during the DMA window,
projection/V/Wo work is spread as background units across the st slots, and
attnV runs through a lagged queue so each phase's first scores beat the
previous phase's last attnV onto the PE queue.
"""

import ml_dtypes
import numpy as np

import concourse.mybir as mybir
import concourse.tile as tile
from concourse import bacc
from concourse.bass_utils import run_bass_kernel_spmd

B, S, D, H = 4, 2048, 512, 8
DH = D // H          # 64
HPC = H // 2         # 4 heads per core
HE = HPC * DH        # 256 output-proj rows per core
T = S
NDC = D // 128       # 4 contraction chunks for projections
NST = S // 128       # 16 key tiles
NTW = T // 512       # 4 query windows
MASK_NUM = 1.0e9
N_CORES = 8

f32 = mybir.dt.float32
bf16 = mybir.dt.bfloat16
f8 = mybir.dt.float8e4
EXP = mybir.ActivationFunctionType.Exp


def build_nc():
    nc = bacc.Bacc("TRN2", target_bir_lowering=False, debug=False, num_devices=1)

    xT = nc.dram_tensor("xT", [D, S], bf16, kind="ExternalInput").ap()
    wq = nc.dram_tensor("wq", [D, HE], bf16, kind="ExternalInput").ap()
    wk = nc.dram_tensor("wk", [D, HE], bf16, kind="ExternalInput").ap()
    wv = nc.dram_tensor("wv", [D, HE], bf16, kind="ExternalInput").ap()
    wo = nc.dram_tensor("wo", [HE, D], bf16, kind="ExternalInput").ap()
    mb = nc.dram_tensor("mbias", [S], f32, kind="ExternalInput").ap()
    y = nc.dram_tensor("y", [T, D], bf16, kind="ExternalOutput").ap()

    with tile.TileContext(nc) as tc:
        with (
            tc.tile_pool(name="const", bufs=1) as const,
            tc.tile_pool(name="psS", bufs=2, space="PSUM") as psS,
            tc.tile_pool(name="psAV", bufs=3, space="PSUM") as psAV,
            tc.tile_pool(name="psM", bufs=1, space="PSUM") as psM,
            tc.tile_pool(name="attnT", bufs=20) as at_pool,
            tc.tile_pool(name="yout", bufs=3) as y_pool,
            tc.tile_pool(name="recip", bufs=4) as r_pool,
            tc.tile_pool(name="recipb", bufs=4) as rb_pool,
        ):
            # ---- ACT table warm-up + PE warm-up source tiles ---------------
            warm_i = const.tile([1, 1], f32, tag="warm_i")
            nc.gpsimd.memset(warm_i[:], 0.0)
            ones33 = const.tile([33, 64], bf16, tag="ones33")
            nc.gpsimd.memset(ones33[:], 1.0)
            ones_f = const.tile([1, 64], f32, tag="ones_f")
            nc.gpsimd.memset(ones_f[:], 1.0)
            warm_o = const.tile([1, 1], f32, tag="warm_o")
            nc.scalar.activation(warm_o[:], warm_i[:], EXP)

            # ---- loads (priority order: first-needed first; x in s-column
            # pieces and wk/wq in d-chunks so the first projection chains can
            # start right as their slices land) --------------------------------
            mb_sb = const.tile([128, NST], f32, tag="mb")
            nc.sync.dma_start(mb_sb[:], mb.rearrange("(j p) -> p j", p=128))
            wk_sb = const.tile([128, NDC, HE], bf16, tag="wk")
            nc.sync.dma_start(wk_sb[:], wk.rearrange("(c p) n -> p c n", p=128))
            wq_sb = const.tile([128, NDC, HE], bf16, tag="wq")
            nc.sync.dma_start(wq_sb[:], wq.rearrange("(c p) n -> p c n", p=128))
            xT_sb = const.tile([128, NDC, S], bf16, tag="xT")
            xT_r = xT.rearrange("(c p) s -> c p s", p=128)

            def dma_x_piece(w):
                for c in range(NDC):
                    nc.sync.dma_start(
                        xT_sb[:, c, w * 512 : (w + 1) * 512],
                        xT_r[c][:, w * 512 : (w + 1) * 512],
                    )

            dma_x_piece(0)
            dma_x_piece(1)
            wv_sb = const.tile([128, NDC, HE], bf16, tag="wv")
            nc.sync.dma_start(wv_sb[:], wv.rearrange("(c p) n -> p c n", p=128))
            dma_x_piece(2)
            dma_x_piece(3)
            wo_sb = const.tile([128, HE // 128, D], bf16, tag="wo")
            nc.sync.dma_start(wo_sb[:], wo.rearrange("(c p) n -> p c n", p=128))

            # V' tiles: [s-tile][local head][DH + ones column]
            v_sb = const.tile([128, NST, HPC, DH + 1], bf16, tag="v")
            nc.gpsimd.memset(v_sb[:, :, :, DH : DH + 1], 1.0)

            kt = [
                const.tile([128, S], bf16, tag=f"kt{pp}", name=f"kt{pp}")
                for pp in range(2)
            ]
            qt = [
                const.tile([128, S], bf16, tag=f"qt{pp}", name=f"qt{pp}")
                for pp in range(2)
            ]
            # outT [he, t] as [128, 2, T]: chunk pp, rows h2*64
            outT_sb = const.tile([128, HE // 128, T], bf16, tag="outT")

            # PE keep-warm: tiny matmuls with no data deps; they run during
            # DMA/normalize waits so the HAM clock gate stays open. The
            # scratch PSUM comes from the attnV ring (slot timing keeps the
            # ring WARs on long-dead tiles).
            def emit_pe_warm(n):
                warm_ps = psAV.tile([64, 64], f32, tag="av", name="warm_ps")
                for _ in range(n):
                    nc.tensor.matmul(
                        warm_ps[:],
                        lhsT=ones33[0:1, :],
                        rhs=ones33[0:1, :],
                        start=True,
                        stop=True,
                    )

            emit_pe_warm(85)

            # ---- background work units -------------------------------------
            kq_done: set[tuple[int, str, int]] = set()
            v_done = [False] * NST

            def emit_kq_chunk(pp, which, sc, pool=None):
                key = (pp, which, sc)
                if key in kq_done:
                    return
                kq_done.add(key)
                w_sb = wk_sb if which == "k" else wq_sb
                dst = kt[pp] if which == "k" else qt[pp]
                if pool is None:
                    ps = psM.tile([128, 512], f32, tag="mm", name="proj_ps")
                else:
                    ps = pool.tile([128, 512], f32, tag="sc", name="proj_ps")
                for dc in range(NDC):
                    nc.tensor.matmul(
                        ps[:],
                        lhsT=w_sb[:, dc, pp * 128 : (pp + 1) * 128],
                        rhs=xT_sb[:, dc, sc * 512 : (sc + 1) * 512],
                        start=(dc == 0),
                        stop=(dc == NDC - 1),
                    )
                nc.vector.tensor_copy(dst[:, sc * 512 : (sc + 1) * 512], ps[:])

            def emit_v_chain(vst):
                if v_done[vst]:
                    return
                v_done[vst] = True
                ps = psM.tile([128, HE], f32, tag="mm", name="v_ps")
                for dc in range(NDC):
                    nc.tensor.matmul(
                        ps[:],
                        lhsT=xT_sb[:, dc, vst * 128 : (vst + 1) * 128],
                        rhs=wv_sb[:, dc, :],
                        start=(dc == 0),
                        stop=(dc == NDC - 1),
                    )
                nc.vector.tensor_copy(
                    v_sb[:, vst, :, 0:DH],
                    ps[:].rearrange("p (h e) -> p h e", e=DH),
                )

            def emit_wo_tt(tt, pool=None):
                if pool is None:
                    ps = psM.tile([128, 512], f32, tag="mm", name="y_ps")
                else:
                    ps = pool.tile([128, 512], f32, tag="sc", name="y_ps")
                for c in range(HE // 128):
                    nc.tensor.matmul(
                        ps[:],
                        lhsT=outT_sb[:, c, tt * 128 : (tt + 1) * 128],
                        rhs=wo_sb[:, c, :],
                        start=(c == 0),
                        stop=(c == HE // 128 - 1),
                    )
                y_sb = y_pool.tile([128, 512], bf16, tag="y", name="y_sb")
                if pool is None or tt % 2 == 0:
                    nc.vector.tensor_copy(y_sb[:], ps[:])
                else:
                    # tail: Scalar engine is idle; split the psum->bf16 copies
                    nc.scalar.copy(y_sb[:], ps[:])
                nc.sync.dma_start(y[tt * 128 : (tt + 1) * 128, :], y_sb[:])

            def emit_normalize(tw_, pp_, av_, tail=False):
                # per-head chain: denom copy -> reciprocal -> broadcast -> mul
                # (muls in column halves so downstream Wo can start earlier;
                # in the tail the Scalar engine is idle, so it takes the
                # denominator copies off the Vector engine's critical chain)
                rbs = []
                for h2 in range(2):
                    r_t = r_pool.tile([1, 512], f32, tag="r", name="r_t")
                    if tail:
                        nc.scalar.copy(r_t[0:1, :], av_[h2][DH : DH + 1, :])
                    else:
                        nc.vector.tensor_copy(r_t[0:1, :], av_[h2][DH : DH + 1, :])
                    ri_t = r_pool.tile([1, 512], f32, tag="ri", name="ri_t")
                    nc.vector.reciprocal_approx_fast(ri_t[0:1, :], r_t[0:1, :])
                    rb_t = rb_pool.tile([64, 512], f32, tag="rb", name="rb_t")
                    nc.gpsimd.partition_broadcast(rb_t[:], ri_t[0:1, :])
                    rbs.append(rb_t)
                for half in range(2):
                    cl, cr = half * 256, (half + 1) * 256
                    for h2 in range(2):
                        nc.vector.tensor_mul(
                            outT_sb[
                                h2 * 64 : (h2 + 1) * 64,
                                pp_,
                                tw_ * 512 + cl : tw_ * 512 + cr,
                            ],
                            av_[h2][0:DH, cl:cr],
                            rbs[h2][:, cl:cr],
                        )

            # unit kinds: ("v", vst) | ("kq", pp, which, sc) | ("wo", tt)
            # Hard data requirements are enforced inline, so ordering here
            # only shapes engine-queue pacing.
            UNIT_NS = {"v": 430, "kq": 900, "wo": 1000}
            bg_by_phase: dict[int, list[tuple]] = {
                0: [("kq", 1, "k", 0), ("kq", 1, "q", 0),
                    ("v", 4), ("v", 5), ("kq", 0, "k", 2), ("v", 6), ("v", 7),
                    ("kq", 0, "k", 3)]
                + [("v", i) for i in range(8, 13)],
                1: [("v", 13), ("kq", 1, "k", 1), ("v", 14),
                    ("kq", 1, "k", 2), ("v", 15),
                    ("kq", 1, "k", 3), ("kq", 0, "q", 1)],
                2: [("kq", 1, "q", 1)],
                3: [("kq", 0, "q", 2),
                    ("wo", 0), ("wo", 2)],
                4: [("kq", 1, "q", 2)],
                5: [("kq", 0, "q", 3),
                    ("wo", 4), ("wo", 6)],
                6: [("kq", 1, "q", 3)],
                7: [("wo", 8), ("wo", 10)],
            }
            SLOT_NS = {0: 760.0, 1: 700.0}

            def run_unit(u):
                if u[0] == "v":
                    emit_v_chain(u[1])
                elif u[0] == "kq":
                    emit_kq_chunk(u[1], u[2], u[3])
                else:
                    emit_wo_tt(u[1])
                    emit_wo_tt(u[1] + 1)

            # ---- prologue: minimum projections for phase 0's first slots ---
            # (scratch from the still-empty scores ring: two banks in flight)
            emit_kq_chunk(0, "k", 0, pool=psS)
            emit_kq_chunk(0, "q", 0, pool=psS)

            # ---- main phase loop -------------------------------------------
            # pend: global attnV queue entries (phase_idx, st, at_tile)
            # av tiles are allocated lazily per phase (after the previous
            # phase's normalize has been emitted) to keep ring WARs legal.
            phases = [(tw, pp) for tw in range(NTW) for pp in range(2)]
            pend: list[tuple[int, int, object]] = []
            av_by_phase: dict[int, list] = {}

            def drain_attnv(cur_p, upto_p, min_keep, max_n=1000):
                """Emit attnV for queued at-tiles of phases <= upto_p, keeping
                at least min_keep entries queued (pipeline lag)."""
                n = 0
                while (
                    pend
                    and len(pend) > min_keep
                    and pend[0][0] <= upto_p
                    and n < max_n
                ):
                    n += 1
                    p_, st_, at__ = pend[0]
                    if not v_done[st_]:
                        if p_ == cur_p:
                            break
                        emit_v_chain(st_)
                    if p_ not in av_by_phase:
                        av_by_phase[p_] = [
                            psAV.tile([DH + 1, 512], f32, tag="av", name=f"av{h2}")
                            for h2 in range(2)
                        ]
                    pend.pop(0)
                    av_ = av_by_phase[p_]
                    pp_ = phases[p_][1]
                    for h2 in range(2):
                        nc.tensor.matmul(
                            av_[h2][0 : DH + 1, :],
                            lhsT=v_sb[:, st_, 2 * pp_ + h2, :],
                            rhs=at__[:, h2 * 512 : (h2 + 1) * 512],
                            start=(st_ == 0),
                            stop=(st_ == NST - 1),
                        )

            def emit_scores_exp(g):
                """Scores + Exp for global slot g = p*NST + st."""
                p_, st_ = g // NST, g % NST
                tw_, pp_ = phases[p_]
                emit_kq_chunk(pp_, "k", st_ // 4)
                emit_kq_chunk(pp_, "q", tw_)
                sc = psS.tile([128, 1024], f32, tag="sc", name="sc")
                with tc.high_priority(offset=40):
                    for h2 in range(2):
                        nc.tensor.matmul(
                            sc[:, h2 * 512 : (h2 + 1) * 512],
                            lhsT=kt[pp_][
                                h2 * 64 : (h2 + 1) * 64, st_ * 128 : (st_ + 1) * 128
                            ],
                            rhs=qt[pp_][
                                h2 * 64 : (h2 + 1) * 64, tw_ * 512 : (tw_ + 1) * 512
                            ],
                            start=True,
                            stop=True,
                        )
                at_t = at_pool.tile([128, 1024], bf16, tag="at", name="at")
                nc.scalar.activation(
                    at_t[:], sc[:], EXP, bias=mb_sb[:, st_ : st_ + 1], scale=0.125
                )
                pend.append((p_, st_, at_t))

            n_slots = len(phases) * NST

            # head-window reclamation: the first two score tiles plus the
            # projection/V chains whose x pieces land during the DMA window
            # are emitted before the phase loop, so the otherwise-idle PE
            # head absorbs work that would crowd phases 0-1
            emit_scores_exp(0)
            emit_scores_exp(1)
            emit_kq_chunk(0, "k", 1)
            for vst in range(4):
                emit_v_chain(vst)
            g_next = 2  # next global slot whose scores/Exp are not yet emitted

            for p, (tw, pp) in enumerate(phases):
                units = list(bg_by_phase[p])
                budget = 0.0
                for st in range(NST):
                    # emit scores one slot ahead so the Exp stream never waits
                    # behind this slot's background/attnV work in engine order
                    g = p * NST + st
                    while g_next <= min(g + 1, n_slots - 1):
                        emit_scores_exp(g_next)
                        g_next += 1

                    if st == 14 and p > 0:
                        # previous phase's attnV has long drained through the
                        # lagged queue; normalize it (frees its av ring slots
                        # for phase p+1's first allocations)
                        drain_attnv(p, p - 1, 0)
                        prev_p = p - 1
                        emit_normalize(*phases[prev_p], av_by_phase[prev_p])

                    # opportunistic background work
                    budget += SLOT_NS.get(p, 450.0)
                    while units and budget >= UNIT_NS[units[0][0]]:
                        if units[0][0] == "wo" and st < 2:
                            break
                        u = units.pop(0)
                        budget -= UNIT_NS[u[0]]
                        run_unit(u)

                    # drain two key-tiles every other slot: the four
                    # back-to-back attnV matmuls hide weight-load boundaries
                    # that per-slot single drains expose
                    if st % 2 == 1:
                        drain_attnv(p, p, 3, max_n=2)

                for u in units:
                    run_unit(u)

            # ---- tail: finish last phase, normalize, last Wo group ---------
            # (keep-warm matmuls run on PE during the normalize chain; Wo
            # double-buffers through the now-free scores ring)
            last = len(phases) - 1
            drain_attnv(last, last, 0)
            emit_normalize(*phases[last], av_by_phase[last], tail=True)
            for tt in range(12, 16):
                emit_wo_tt(tt, pool=psS)

    nc.compile()
    return nc


_NC_CACHE = None


def _get_nc():
    global _NC_CACHE
    if _NC_CACHE is None:
        _NC_CACHE = build_nc()
    return _NC_CACHE


def make_in_maps(x, mask, Wq, Wk, Wv, Wo):
    bf = ml_dtypes.bfloat16
    xT = np.ascontiguousarray(x.transpose(0, 2, 1)).astype(bf)  # [B, D, S]
    # [H, D, DH] -> [D, H*DH]
    wq_f = np.ascontiguousarray(Wq.transpose(1, 0, 2).reshape(D, H * DH))
    wk_f = np.ascontiguousarray(Wk.transpose(1, 0, 2).reshape(D, H * DH))
    wv_f = np.ascontiguousarray(Wv.transpose(1, 0, 2).reshape(D, H * DH))
    mb = np.where(mask > 0, 0.0, -MASK_NUM).astype(np.float32)  # [B, S]
    in_maps = []
    for c in range(N_CORES):
        b, hg = c // 2, c % 2
        cols = slice(hg * HE, (hg + 1) * HE)
        in_maps.append(
            {
                "xT": xT[b],
                "wq": np.ascontiguousarray(wq_f[:, cols]).astype(bf),
                "wk": np.ascontiguousarray(wk_f[:, cols]).astype(bf),
                "wv": np.ascontiguousarray(wv_f[:, cols]).astype(bf),
                "wo": np.ascontiguousarray(Wo[cols, :]).astype(bf),
                "mbias": mb[b],
            }
        )
    return in_maps


def combine_results(results):
    y = np.zeros((B, S, D), np.float32)
    for c in range(N_CORES):
        y[c // 2] += results[c]["y"].astype(np.float32)
    return y


def kernel(x, mask, Wq, Wk, Wv, Wo):
    nc = _get_nc()
    in_maps = make_in_maps(
        np.asarray(x, np.float32),
        np.asarray(mask),
        np.asarray(Wq, np.float32),
        np.asarray(Wk, np.float32),
        np.asarray(Wv, np.float32),
        np.asarray(Wo, np.float32),
    )
    res = run_bass_kernel_spmd(nc, in_maps, core_ids=list(range(N_CORES)))
    return combine_results(res.results)



# revision 14
# speedup vs baseline: 1.0787x; 1.0787x over previous
"""Trainium2 Bass kernel for nn_Encoder (B=4, S=2048, D=512, H=8 self-attention).

Sharding over 8 NeuronCores: core c -> (batch b = c//2, head-group hg = c%2).
Each core computes, for its batch and its 4 heads, the full attention block
plus a partial output projection y_part = attn_out @ Wo[group rows]. The host
sums the two partial y tensors per batch.

v3: host-side key compaction. The key-padding mask kills ~half the keys
(exp(-1e9) == 0), so the host gathers only the valid key columns of x and
pads to SK = NKT*128 slots (NKT=9 for the ~1044-valid-key regime). Pad
slots are zero columns: their scores are exactly 0, so exp gives exactly
1.0, and they are excluded exactly by (a) V' pad rows being zero (numerator)
and (b) the denominator "ones column" of V' holding the validity mask
instead of all-ones. No attention bias input is needed at all.

The Scalar engine Exp stream is the roofline (~75us for 8 phases). Scores
for (key-tile, head) chunks land in two ping-pong PSUM pools (3 banks +
2 banks) so each ACTIVATE covers N=1536/N=1024 elements, amortizing the
~220-cycle instruction overhead while leaving 3 PSUM banks for the attnV
accumulators ([65, 1024]: dh rows + denominator row, 2 heads) and one for
projection scratch. Score matmuls for the two heads of a pair alternate
row groups 0-63/64-127 so they run concurrently on the PE sub-arrays.
"""

import math

import ml_dtypes
import numpy as np

import concourse.mybir as mybir
import concourse.tile as tile
from concourse import bacc
from concourse.bass_utils import run_bass_kernel_spmd

B, S, D, H = 4, 2048, 512, 8
DH = D // H          # 64
HPC = H // 2         # 4 heads per core
HE = HPC * DH        # 256 output-proj rows per core
T = S
NDC = D // 128       # 4 contraction chunks for projections
NTW = T // 512       # 4 query windows
N_CORES = 8
NKT_DEFAULT = 9      # key tiles (128 keys each) after compaction

f32 = mybir.dt.float32
bf16 = mybir.dt.bfloat16
EXP = mybir.ActivationFunctionType.Exp


def _group_sizes(n_chunks):
    """Split n_chunks score chunks into ACT groups alternating 3/2 wide."""
    sizes = []
    want = 3
    rem = n_chunks
    while rem > 0:
        take = min(want, rem)
        sizes.append(take)
        rem -= take
        want = 2 if want == 3 else 3
    return sizes


def build_nc(nkt=NKT_DEFAULT):
    SK = nkt * 128
    nc = bacc.Bacc("TRN2", target_bir_lowering=False, debug=False, num_devices=1)

    xq = nc.dram_tensor("xq", [D, S], bf16, kind="ExternalInput").ap()
    xk = nc.dram_tensor("xk", [D, SK], bf16, kind="ExternalInput").ap()
    wq = nc.dram_tensor("wq", [D, HE], bf16, kind="ExternalInput").ap()
    wk = nc.dram_tensor("wk", [D, HE], bf16, kind="ExternalInput").ap()
    wv = nc.dram_tensor("wv", [D, HE], bf16, kind="ExternalInput").ap()
    wo = nc.dram_tensor("wo", [HE, D], bf16, kind="ExternalInput").ap()
    vmask = nc.dram_tensor("vmask", [128, nkt * HPC], bf16, kind="ExternalInput").ap()
    y = nc.dram_tensor("y", [T, D], bf16, kind="ExternalOutput").ap()

    # K-piece boundaries for the K' projection (rhs free <= 512)
    kp = []
    off = 0
    while off < SK:
        w = min(512, SK - off)
        kp.append((off, w))
        off += w

    n_chunks = 2 * nkt
    gsizes = _group_sizes(n_chunks)

    with tile.TileContext(nc) as tc:
        with (
            tc.tile_pool(name="const", bufs=1) as const,
            tc.tile_pool(name="psA", bufs=1, space="PSUM") as psA,
            tc.tile_pool(name="psB", bufs=1, space="PSUM") as psB,
            tc.tile_pool(name="psAV", bufs=2, space="PSUM") as psAV,
            tc.tile_pool(name="psM", bufs=1, space="PSUM") as psM,
            tc.tile_pool(name="atp", bufs=8) as at_pool,
            tc.tile_pool(name="yout", bufs=3) as y_pool,
            tc.tile_pool(name="recip", bufs=4) as r_pool,
            tc.tile_pool(name="recipb", bufs=4) as rb_pool,
        ):
            # ---- ACT table warm-up + PE warm-up source tiles ---------------
            warm_i = const.tile([1, 1], f32, tag="warm_i")
            nc.gpsimd.memset(warm_i[:], 0.0)
            ones33 = const.tile([33, 64], bf16, tag="ones33")
            nc.gpsimd.memset(ones33[:], 1.0)
            warm_o = const.tile([1, 1], f32, tag="warm_o")
            nc.scalar.activation(warm_o[:], warm_i[:], EXP)

            # ---- loads (priority order: first-needed first) ----------------
            wk_sb = const.tile([128, NDC, HE], bf16, tag="wk")
            nc.sync.dma_start(wk_sb[:], wk.rearrange("(c p) n -> p c n", p=128))
            xk_sb = const.tile([128, NDC, SK], bf16, tag="xk")
            xk_r = xk.rearrange("(c p) s -> c p s", p=128)

            def dma_xk_piece(off, w):
                for c in range(NDC):
                    nc.sync.dma_start(
                        xk_sb[:, c, off : off + w], xk_r[c][:, off : off + w]
                    )

            dma_xk_piece(0, 512)
            wq_sb = const.tile([128, NDC, HE], bf16, tag="wq")
            nc.sync.dma_start(wq_sb[:], wq.rearrange("(c p) n -> p c n", p=128))
            xq_sb = const.tile([128, NDC, S], bf16, tag="xq")
            xq_r = xq.rearrange("(c p) s -> c p s", p=128)

            def dma_xq_piece(qw):
                for c in range(NDC):
                    nc.sync.dma_start(
                        xq_sb[:, c, qw * 512 : (qw + 1) * 512],
                        xq_r[c][:, qw * 512 : (qw + 1) * 512],
                    )

            dma_xq_piece(0)
            dma_xk_piece(512, min(512, SK - 512))
            wv_sb = const.tile([128, NDC, HE], bf16, tag="wv")
            nc.sync.dma_start(wv_sb[:], wv.rearrange("(c p) n -> p c n", p=128))
            if SK > 1024:
                dma_xk_piece(1024, SK - 1024)

            # V' tiles: [partition(key in tile)][key-tile][local head][DH + vcol]
            # column DH holds the validity mask (1 valid / 0 pad) so the
            # denominator row of attnV excludes pad keys exactly.
            v_sb = const.tile([128, nkt, HPC, DH + 1], bf16, tag="v")
            nc.sync.dma_start(
                v_sb[:, :, :, DH],
                vmask.rearrange("p (j h) -> p j h", h=HPC),
            )
            dma_xq_piece(1)
            dma_xq_piece(2)
            dma_xq_piece(3)
            wo_sb = const.tile([128, HE // 128, D], bf16, tag="wo")
            nc.sync.dma_start(wo_sb[:], wo.rearrange("(c p) n -> p c n", p=128))

            kt = [
                const.tile([128, SK], bf16, tag=f"kt{pp}", name=f"kt{pp}")
                for pp in range(2)
            ]
            qt = [
                const.tile([128, S], bf16, tag=f"qt{pp}", name=f"qt{pp}")
                for pp in range(2)
            ]
            # outT [he, t] as [128, 2, T]: chunk pp, rows h2*64
            outT_sb = const.tile([128, HE // 128, T], bf16, tag="outT")

            # PE keep-warm: tiny matmuls with no data deps run during the
            # DMA window so the HAM clock gate stays open.
            def emit_pe_warm(n):
                warm_ps = psAV.tile([64, 64], f32, tag="av", name="warm_ps")
                for _ in range(n):
                    nc.tensor.matmul(
                        warm_ps[:],
                        lhsT=ones33[0:1, :],
                        rhs=ones33[0:1, :],
                        start=True,
                        stop=True,
                    )

            emit_pe_warm(0)

            # ---- background work units -------------------------------------
            kq_done: set[tuple] = set()
            v_done = [False] * nkt

            def emit_k_piece(pp, pi, pool=None):
                key = ("k", pp, pi)
                if key in kq_done:
                    return
                kq_done.add(key)
                off, w = kp[pi]
                ps = (pool or psM).tile(
                    [128, 512], f32, tag="mm" if pool is None else "sc", name="k_ps"
                )
                for dc in range(NDC):
                    nc.tensor.matmul(
                        ps[:, 0:w],
                        lhsT=wk_sb[:, dc, pp * 128 : (pp + 1) * 128],
                        rhs=xk_sb[:, dc, off : off + w],
                        start=(dc == 0),
                        stop=(dc == NDC - 1),
                    )
                nc.vector.tensor_copy(kt[pp][:, off : off + w], ps[:, 0:w])

            def emit_q_piece(pp, qw, pool=None):
                key = ("q", pp, qw)
                if key in kq_done:
                    return
                kq_done.add(key)
                ps = (pool or psM).tile(
                    [128, 512], f32, tag="mm" if pool is None else "sc", name="q_ps"
                )
                for dc in range(NDC):
                    nc.tensor.matmul(
                        ps[:],
                        lhsT=wq_sb[:, dc, pp * 128 : (pp + 1) * 128],
                        rhs=xq_sb[:, dc, qw * 512 : (qw + 1) * 512],
                        start=(dc == 0),
                        stop=(dc == NDC - 1),
                    )
                nc.vector.tensor_copy(qt[pp][:, qw * 512 : (qw + 1) * 512], ps[:])

            def emit_v_chain(vst, pool=None):
                if v_done[vst]:
                    return
                v_done[vst] = True
                ps = (pool or psM).tile(
                    [128, HE], f32, tag="mm" if pool is None else "sc", name="v_ps"
                )
                for dc in range(NDC):
                    nc.tensor.matmul(
                        ps[:],
                        lhsT=xk_sb[:, dc, vst * 128 : (vst + 1) * 128],
                        rhs=wv_sb[:, dc, :],
                        start=(dc == 0),
                        stop=(dc == NDC - 1),
                    )
                nc.vector.tensor_copy(
                    v_sb[:, vst, :, 0:DH],
                    ps[:].rearrange("p (h e) -> p h e", e=DH),
                )

            def emit_wo_tt(tt, pool=None):
                ps = (pool or psM).tile(
                    [128, 512], f32, tag="mm" if pool is None else "sc", name="y_ps"
                )
                for c in range(HE // 128):
                    nc.tensor.matmul(
                        ps[:],
                        lhsT=outT_sb[:, c, tt * 128 : (tt + 1) * 128],
                        rhs=wo_sb[:, c, :],
                        start=(c == 0),
                        stop=(c == HE // 128 - 1),
                    )
                y_sb = y_pool.tile([128, 512], bf16, tag="y", name="y_sb")
                nc.vector.tensor_copy(y_sb[:], ps[:])
                nc.sync.dma_start(y[tt * 128 : (tt + 1) * 128, :], y_sb[:])

            def run_unit(u):
                if u[0] == "v":
                    emit_v_chain(u[1])
                elif u[0] == "k":
                    emit_k_piece(u[1], u[2])
                elif u[0] == "q":
                    emit_q_piece(u[1], u[2])
                else:
                    emit_wo_tt(u[1])

            # phases: qw-major, pp-inner so Wo(qw) unblocks early
            phases = [(qw, pp) for qw in range(NTW) for pp in range(2)]

            # chunk c of a phase -> (key tile, head-in-pair)
            # groups partition the 2*nkt chunks into ACT-sized pieces
            groups = []
            c0 = 0
            for gs in gsizes:
                groups.append(list(range(c0, c0 + gs)))
                c0 += gs

            pend: list[tuple] = []   # (phase_idx, at_tile, chunk_list)
            av_by_phase: dict[int, object] = {}

            def emit_scores_group(p, gi):
                qw, pp = phases[p]
                chunks = groups[gi]
                pool = psA if gsizes[gi] == 3 else psB
                width = gsizes[gi] * 512
                sc = pool.tile([128, width], f32, tag="sc", name="sc")
                with tc.high_priority(offset=40):
                    for i, c in enumerate(chunks):
                        ktile, h2 = c // 2, c % 2
                        nc.tensor.matmul(
                            sc[:, i * 512 : (i + 1) * 512],
                            lhsT=kt[pp][
                                h2 * 64 : (h2 + 1) * 64,
                                ktile * 128 : (ktile + 1) * 128,
                            ],
                            rhs=qt[pp][
                                h2 * 64 : (h2 + 1) * 64, qw * 512 : (qw + 1) * 512
                            ],
                            start=True,
                            stop=True,
                        )
                at_t = at_pool.tile([128, width], bf16, tag="at", name="at")
                nc.scalar.activation(at_t[:], sc[:], EXP, scale=0.125)
                pend.append((p, at_t, chunks))

            def drain_attnv(upto_p, min_keep, max_n=1000):
                n = 0
                while pend and len(pend) > min_keep and pend[0][0] <= upto_p and n < max_n:
                    n += 1
                    p_, at_t, chunks = pend.pop(0)
                    qw_, pp_ = phases[p_]
                    for c in chunks:
                        if not v_done[c // 2]:
                            emit_v_chain(c // 2)
                    if p_ not in av_by_phase:
                        av_by_phase[p_] = [
                            psAV.tile([DH + 1, 512], f32, tag="av", name=f"av{h2}")
                            for h2 in range(2)
                        ]
                    av_ = av_by_phase[p_]
                    for i, c in enumerate(chunks):
                        ktile, h2 = c // 2, c % 2
                        nc.tensor.matmul(
                            av_[h2][:],
                            lhsT=v_sb[:, ktile, 2 * pp_ + h2, :],
                            rhs=at_t[:, i * 512 : (i + 1) * 512],
                            start=(ktile == 0),
                            stop=(ktile == nkt - 1),
                        )

            def emit_normalize(p):
                qw_, pp_ = phases[p]
                av_ = av_by_phase[p]
                for h2 in range(2):
                    # denominator row must be copied to a partition-0 SBUF
                    # tile first: the custom-DVE reciprocal cannot read at
                    # partition offset 64 (baseline-proven chain)
                    rt = r_pool.tile([1, 512], f32, tag="rt", name="rt")
                    nc.vector.tensor_copy(rt[0:1, :], av_[h2][DH : DH + 1, :])
                    ri = r_pool.tile([1, 512], f32, tag="ri", name="ri")
                    nc.vector.reciprocal_approx_fast(ri[0:1, :], rt[0:1, :])
                    rb = rb_pool.tile([64, 512], f32, tag="rb", name="rb")
                    nc.gpsimd.partition_broadcast(rb[:], ri[0:1, :])
                    nc.vector.tensor_mul(
                        outT_sb[
                            h2 * 64 : (h2 + 1) * 64,
                            pp_,
                            qw_ * 512 : (qw_ + 1) * 512,
                        ],
                        av_[h2][0:DH, :],
                        rb[:],
                    )

            # background units per phase (hard deps enforced by Tile; this
            # ordering shapes engine-queue pacing and respects DMA arrival).
            # Phase p's K'/Q' inputs must be emitted during phase p-1 at the
            # latest so the strict-FIFO PE queue never stalls on them.
            # phase 0: K' pieces for pp0 must be emitted before the scores
            # chunks that read them (program order IS the data, Tile does
            # not reorder a read ahead of a later write); V' chains before
            # the lagged attnV drains; pp1's K'/Q' before phase 1.
            ph0 = [("q", 1, 0)]
            if len(kp) > 1:
                ph0.append(("k", 0, 1))
            ph0 += [("v", 0), ("v", 1)]
            if len(kp) > 2:
                ph0.append(("k", 0, 2))
            ph0.append(("k", 1, 0))
            ph0.append(("v", 2))
            if len(kp) > 1:
                ph0.append(("k", 1, 1))
            ph0.append(("v", 3))
            if len(kp) > 2:
                ph0.append(("k", 1, 2))
            ph0 += [("v", i) for i in range(4, nkt)]
            ph1 = [("q", 0, 1)]
            bg_by_phase = {
                0: ph0,
                1: ph1,
                2: [("q", 1, 1), ("wo", 0), ("wo", 1)],
                3: [("q", 0, 2), ("wo", 2), ("wo", 3)],
                4: [("q", 1, 2), ("wo", 4), ("wo", 5)],
                5: [("q", 0, 3), ("wo", 6), ("wo", 7)],
                6: [("q", 1, 3), ("wo", 8), ("wo", 9)],
                7: [("wo", 10), ("wo", 11)],
            }

            # ---- prologue: minimum inputs for phase 0's first groups -------
            emit_k_piece(0, 0, pool=psA)
            emit_q_piece(0, 0, pool=psB)

            # ---- main phase loop -------------------------------------------
            for p in range(len(phases)):
                units = list(bg_by_phase.get(p, []))
                for gi in range(len(groups)):
                    emit_scores_group(p, gi)
                    if p > 0 and gi == 1:
                        # finish previous phase: attnV remainder + normalize
                        drain_attnv(p - 1, 0)
                        emit_normalize(p - 1)
                    if units:
                        run_unit(units.pop(0))
                    drain_attnv(p, 2, max_n=1)
                    if gi >= 4:
                        drain_attnv(p, 2, max_n=1)
                for u in units:
                    run_unit(u)

            # ---- tail: finish last phase, normalize, last Wo group ---------
            last = len(phases) - 1
            drain_attnv(last, 0)
            emit_normalize(last)
            emit_wo_tt(12, pool=psA)
            emit_wo_tt(13, pool=psB)
            emit_wo_tt(14)
            emit_wo_tt(15, pool=psA)

    nc.compile()
    return nc


_NC_CACHE: dict[int, object] = {}


def _get_nc(nkt=NKT_DEFAULT):
    if nkt not in _NC_CACHE:
        _NC_CACHE[nkt] = build_nc(nkt)
    return _NC_CACHE[nkt]


def make_in_maps(x, mask, Wq, Wk, Wv, Wo, nkt=None):
    bf = ml_dtypes.bfloat16
    mask = np.asarray(mask)
    counts = (mask > 0).sum(axis=1)
    if nkt is None:
        nkt = max(1, int(math.ceil(counts.max() / 128)))
    SK = nkt * 128

    xqT = np.ascontiguousarray(x.transpose(0, 2, 1)).astype(bf)  # [B, D, S]
    # [H, D, DH] -> [D, H*DH]
    wq_f = np.ascontiguousarray(Wq.transpose(1, 0, 2).reshape(D, H * DH))
    wk_f = np.ascontiguousarray(Wk.transpose(1, 0, 2).reshape(D, H * DH))
    wv_f = np.ascontiguousarray(Wv.transpose(1, 0, 2).reshape(D, H * DH))

    xkT = []
    vmasks = []
    for b in range(B):
        idx = np.flatnonzero(mask[b] > 0)
        nv = len(idx)
        xk_b = np.zeros((SK, D), np.float32)
        xk_b[:nv] = x[b][idx]
        xkT.append(np.ascontiguousarray(xk_b.T).astype(bf))
        vm = np.zeros((128, nkt, HPC), np.float32)
        slot = np.arange(nkt * 128).reshape(nkt, 128)
        vm[:, :, :] = (slot.T[:, :, None] < nv).astype(np.float32)
        vmasks.append(vm.reshape(128, nkt * HPC).astype(bf))

    in_maps = []
    for c in range(N_CORES):
        b, hg = c // 2, c % 2
        cols = slice(hg * HE, (hg + 1) * HE)
        in_maps.append(
            {
                "xq": xqT[b],
                "xk": xkT[b],
                "wq": np.ascontiguousarray(wq_f[:, cols]).astype(bf),
                "wk": np.ascontiguousarray(wk_f[:, cols]).astype(bf),
                "wv": np.ascontiguousarray(wv_f[:, cols]).astype(bf),
                "wo": np.ascontiguousarray(Wo[cols, :]).astype(bf),
                "vmask": vmasks[b],
            }
        )
    return in_maps, nkt


def combine_results(results):
    y = np.zeros((B, S, D), np.float32)
    for c in range(N_CORES):
        y[c // 2] += results[c]["y"].astype(np.float32)
    return y


def kernel(x, mask, Wq, Wk, Wv, Wo):
    in_maps, nkt = make_in_maps(
        np.asarray(x, np.float32),
        np.asarray(mask),
        np.asarray(Wq, np.float32),
        np.asarray(Wk, np.float32),
        np.asarray(Wv, np.float32),
        np.asarray(Wo, np.float32),
    )
    nc = _get_nc(nkt)
    res = run_bass_kernel_spmd(nc, in_maps, core_ids=list(range(N_CORES)))
    return combine_results(res.results)


# revision 17
# speedup vs baseline: 1.2532x; 1.1618x over previous
"""Trainium2 Bass kernel for nn_Encoder (B=4, S=2048, D=512, H=8 self-attention).

Sharding over 8 NeuronCores: core c -> (batch b = c//2, head-group hg = c%2).
Each core computes, for its batch and its 4 heads, the full attention block
plus a partial output projection y_part = attn_out @ Wo[group rows]. The host
sums the two partial y tensors per batch.

v3: host-side key compaction. The key-padding mask kills ~half the keys
(exp(-1e9) == 0), so the host gathers only the valid key columns of x and
pads to SK = NKT*128 slots (NKT=9 for the ~1044-valid-key regime). Pad
slots are zero columns: their scores are exactly 0, so exp gives exactly
1.0, and they are excluded exactly by (a) V' pad rows being zero (numerator)
and (b) the denominator "ones column" of V' holding the validity mask
instead of all-ones. No attention bias input is needed at all.

The Scalar engine Exp stream is the roofline (~75us for 8 phases). Scores
for (key-tile, head) chunks land in two ping-pong PSUM pools (3 banks +
2 banks) so each ACTIVATE covers N=1536/N=1024 elements, amortizing the
~220-cycle instruction overhead while leaving 3 PSUM banks for the attnV
accumulators ([65, 1024]: dh rows + denominator row, 2 heads) and one for
projection scratch. Score matmuls for the two heads of a pair alternate
row groups 0-63/64-127 so they run concurrently on the PE sub-arrays.
"""

import math

import ml_dtypes
import numpy as np

import concourse.mybir as mybir
import concourse.tile as tile
from concourse import bacc
from concourse.bass_utils import run_bass_kernel_spmd

B, S, D, H = 4, 2048, 512, 8
DH = D // H          # 64
HPC = H // 2         # 4 heads per core
HE = HPC * DH        # 256 output-proj rows per core
T = S
NDC = D // 128       # 4 contraction chunks for projections
NTW = T // 512       # 4 query windows
N_CORES = 8
NKT_DEFAULT = 9      # key tiles (128 keys each) after compaction

f32 = mybir.dt.float32
bf16 = mybir.dt.bfloat16
EXP = mybir.ActivationFunctionType.Exp


def _group_sizes(n_chunks):
    """Split n_chunks score chunks into ACT groups alternating 3/2 wide."""
    sizes = []
    want = 3
    rem = n_chunks
    while rem > 0:
        take = min(want, rem)
        sizes.append(take)
        rem -= take
        want = 2 if want == 3 else 3
    return sizes


def build_nc(nkt=NKT_DEFAULT):
    SK = nkt * 128
    nc = bacc.Bacc("TRN2", target_bir_lowering=False, debug=False, num_devices=1)

    xq = nc.dram_tensor("xq", [D, S], bf16, kind="ExternalInput").ap()
    xk = nc.dram_tensor("xk", [D, SK], bf16, kind="ExternalInput").ap()
    wq = nc.dram_tensor("wq", [D, HE], bf16, kind="ExternalInput").ap()
    wk = nc.dram_tensor("wk", [D, HE], bf16, kind="ExternalInput").ap()
    wv = nc.dram_tensor("wv", [D, HE], bf16, kind="ExternalInput").ap()
    wo = nc.dram_tensor("wo", [HE, D], bf16, kind="ExternalInput").ap()
    vmask = nc.dram_tensor("vmask", [128, nkt * HPC], bf16, kind="ExternalInput").ap()
    y = nc.dram_tensor("y", [T, D], bf16, kind="ExternalOutput").ap()

    # K-piece boundaries for the K' projection (rhs free <= 512)
    kp = []
    off = 0
    while off < SK:
        w = min(512, SK - off)
        kp.append((off, w))
        off += w

    n_chunks = 2 * nkt
    gsizes = _group_sizes(n_chunks)

    with tile.TileContext(nc) as tc:
        with (
            tc.tile_pool(name="const", bufs=1) as const,
            tc.tile_pool(name="psA", bufs=1, space="PSUM") as psA,
            tc.tile_pool(name="psB", bufs=1, space="PSUM") as psB,
            tc.tile_pool(name="psAV", bufs=2, space="PSUM") as psAV,
            tc.tile_pool(name="psM", bufs=1, space="PSUM") as psM,
            tc.tile_pool(name="atp", bufs=8) as at_pool,
            tc.tile_pool(name="yout", bufs=3) as y_pool,
            tc.tile_pool(name="recip", bufs=4) as r_pool,
            tc.tile_pool(name="recipb", bufs=4) as rb_pool,
        ):
            # ---- ACT table warm-up + PE warm-up source tiles ---------------
            warm_i = const.tile([1, 1], f32, tag="warm_i")
            nc.gpsimd.memset(warm_i[:], 0.0)
            ones33 = const.tile([33, 64], bf16, tag="ones33")
            nc.gpsimd.memset(ones33[:], 1.0)
            warm_o = const.tile([1, 1], f32, tag="warm_o")
            nc.scalar.activation(warm_o[:], warm_i[:], EXP)

            # ---- loads (priority order: first-needed first) ----------------
            wk_sb = const.tile([128, NDC, HE], bf16, tag="wk")
            nc.sync.dma_start(wk_sb[:], wk.rearrange("(c p) n -> p c n", p=128))
            xk_sb = const.tile([128, NDC, SK], bf16, tag="xk")
            xk_r = xk.rearrange("(c p) s -> c p s", p=128)

            xk_r2 = xk.rearrange("(c p) s -> p c s", p=128)

            def dma_xk_piece(off, w):
                nc.sync.dma_start(
                    xk_sb[:, :, off : off + w], xk_r2[:, :, off : off + w]
                )

            dma_xk_piece(0, 512)
            wq_sb = const.tile([128, NDC, HE], bf16, tag="wq")
            nc.sync.dma_start(wq_sb[:], wq.rearrange("(c p) n -> p c n", p=128))
            xq_sb = const.tile([128, NDC, S], bf16, tag="xq")
            xq_r = xq.rearrange("(c p) s -> c p s", p=128)

            xq_r2 = xq.rearrange("(c p) s -> p c s", p=128)

            def dma_xq_piece(qw):
                nc.sync.dma_start(
                    xq_sb[:, :, qw * 512 : (qw + 1) * 512],
                    xq_r2[:, :, qw * 512 : (qw + 1) * 512],
                )

            dma_xq_piece(0)
            dma_xk_piece(512, min(512, SK - 512))
            wv_sb = const.tile([128, NDC, HE], bf16, tag="wv")
            nc.sync.dma_start(wv_sb[:], wv.rearrange("(c p) n -> p c n", p=128))
            if SK > 1024:
                dma_xk_piece(1024, SK - 1024)

            # V' tiles: [partition(key in tile)][key-tile][local head][DH + vcol]
            # column DH holds the validity mask (1 valid / 0 pad) so the
            # denominator row of attnV excludes pad keys exactly.
            v_sb = const.tile([128, nkt, HPC, DH + 1], bf16, tag="v")
            nc.sync.dma_start(
                v_sb[:, :, :, DH],
                vmask.rearrange("p (j h) -> p j h", h=HPC),
            )
            dma_xq_piece(1)
            dma_xq_piece(2)
            dma_xq_piece(3)
            wo_sb = const.tile([128, HE // 128, D], bf16, tag="wo")
            nc.sync.dma_start(wo_sb[:], wo.rearrange("(c p) n -> p c n", p=128))

            kt = [
                const.tile([128, SK], bf16, tag=f"kt{pp}", name=f"kt{pp}")
                for pp in range(2)
            ]
            qt = [
                const.tile([128, S], bf16, tag=f"qt{pp}", name=f"qt{pp}")
                for pp in range(2)
            ]
            # outT [he, t] as [128, 2, T]: chunk pp, rows h2*64
            outT_sb = const.tile([128, HE // 128, T], bf16, tag="outT")

            # PE keep-warm: tiny matmuls with no data deps run during the
            # DMA window so the HAM clock gate stays open.
            def emit_pe_warm(n):
                warm_ps = psAV.tile([64, 64], f32, tag="av", name="warm_ps")
                for _ in range(n):
                    nc.tensor.matmul(
                        warm_ps[:],
                        lhsT=ones33[0:1, :],
                        rhs=ones33[0:1, :],
                        start=True,
                        stop=True,
                    )

            emit_pe_warm(70)

            # ---- background work units -------------------------------------
            kq_done: set[tuple] = set()
            v_done = [False] * nkt

            def emit_k_piece(pp, pi, pool=None):
                key = ("k", pp, pi)
                if key in kq_done:
                    return
                kq_done.add(key)
                off, w = kp[pi]
                ps = (pool or psM).tile(
                    [128, 512], f32, tag="mm" if pool is None else "sc", name="k_ps"
                )
                for dc in range(NDC):
                    nc.tensor.matmul(
                        ps[:, 0:w],
                        lhsT=wk_sb[:, dc, pp * 128 : (pp + 1) * 128],
                        rhs=xk_sb[:, dc, off : off + w],
                        start=(dc == 0),
                        stop=(dc == NDC - 1),
                    )
                nc.vector.tensor_copy(kt[pp][:, off : off + w], ps[:, 0:w])

            def emit_q_piece(pp, qw, pool=None):
                key = ("q", pp, qw)
                if key in kq_done:
                    return
                kq_done.add(key)
                ps = (pool or psM).tile(
                    [128, 512], f32, tag="mm" if pool is None else "sc", name="q_ps"
                )
                for dc in range(NDC):
                    nc.tensor.matmul(
                        ps[:],
                        lhsT=wq_sb[:, dc, pp * 128 : (pp + 1) * 128],
                        rhs=xq_sb[:, dc, qw * 512 : (qw + 1) * 512],
                        start=(dc == 0),
                        stop=(dc == NDC - 1),
                    )
                nc.vector.tensor_copy(qt[pp][:, qw * 512 : (qw + 1) * 512], ps[:])

            def emit_v_chain(vst, pool=None):
                if v_done[vst]:
                    return
                v_done[vst] = True
                ps = (pool or psM).tile(
                    [128, HE], f32, tag="mm" if pool is None else "sc", name="v_ps"
                )
                for dc in range(NDC):
                    nc.tensor.matmul(
                        ps[:],
                        lhsT=xk_sb[:, dc, vst * 128 : (vst + 1) * 128],
                        rhs=wv_sb[:, dc, :],
                        start=(dc == 0),
                        stop=(dc == NDC - 1),
                    )
                nc.vector.tensor_copy(
                    v_sb[:, vst, :, 0:DH],
                    ps[:].rearrange("p (h e) -> p h e", e=DH),
                )

            def emit_wo_tt(tt, pool=None):
                ps = (pool or psM).tile(
                    [128, 512], f32, tag="mm" if pool is None else "sc", name="y_ps"
                )
                for c in range(HE // 128):
                    nc.tensor.matmul(
                        ps[:],
                        lhsT=outT_sb[:, c, tt * 128 : (tt + 1) * 128],
                        rhs=wo_sb[:, c, :],
                        start=(c == 0),
                        stop=(c == HE // 128 - 1),
                    )
                y_sb = y_pool.tile([128, 512], bf16, tag="y", name="y_sb")
                nc.vector.tensor_copy(y_sb[:], ps[:])
                nc.gpsimd.dma_start(y[tt * 128 : (tt + 1) * 128, :], y_sb[:])

            def run_unit(u):
                if u[0] == "v":
                    emit_v_chain(u[1])
                elif u[0] == "k":
                    emit_k_piece(u[1], u[2])
                elif u[0] == "q":
                    emit_q_piece(u[1], u[2])
                else:
                    emit_wo_tt(u[1])

            # phases: qw-major, pp-inner so Wo(qw) unblocks early
            phases = [(qw, pp) for qw in range(NTW) for pp in range(2)]

            # chunk c of a phase -> (key tile, head-in-pair)
            # groups partition the 2*nkt chunks into ACT-sized pieces
            groups = []
            c0 = 0
            for gs in gsizes:
                groups.append(list(range(c0, c0 + gs)))
                c0 += gs

            pend: list[tuple] = []   # (phase_idx, at_tile, chunk_list)
            av_by_phase: dict[int, object] = {}

            def emit_scores_group(p, gi):
                qw, pp = phases[p]
                chunks = groups[gi]
                pool = psA if gsizes[gi] == 3 else psB
                width = gsizes[gi] * 512
                sc = pool.tile([128, width], f32, tag="sc", name="sc")
                with tc.high_priority(offset=40):
                    for i, c in enumerate(chunks):
                        ktile, h2 = c // 2, c % 2
                        nc.tensor.matmul(
                            sc[:, i * 512 : (i + 1) * 512],
                            lhsT=kt[pp][
                                h2 * 64 : (h2 + 1) * 64,
                                ktile * 128 : (ktile + 1) * 128,
                            ],
                            rhs=qt[pp][
                                h2 * 64 : (h2 + 1) * 64, qw * 512 : (qw + 1) * 512
                            ],
                            start=True,
                            stop=True,
                        )
                at_t = at_pool.tile([128, width], bf16, tag="at", name="at")
                nc.scalar.activation(at_t[:], sc[:], EXP, scale=0.125)
                pend.append((p, at_t, chunks))

            def drain_attnv(upto_p, min_keep, max_n=1000):
                n = 0
                while pend and len(pend) > min_keep and pend[0][0] <= upto_p and n < max_n:
                    n += 1
                    p_, at_t, chunks = pend.pop(0)
                    qw_, pp_ = phases[p_]
                    for c in chunks:
                        if not v_done[c // 2]:
                            emit_v_chain(c // 2)
                    if p_ not in av_by_phase:
                        av_by_phase[p_] = [
                            psAV.tile([DH + 1, 512], f32, tag="av", name=f"av{h2}")
                            for h2 in range(2)
                        ]
                    av_ = av_by_phase[p_]
                    for i, c in enumerate(chunks):
                        ktile, h2 = c // 2, c % 2
                        nc.tensor.matmul(
                            av_[h2][:],
                            lhsT=v_sb[:, ktile, 2 * pp_ + h2, :],
                            rhs=at_t[:, i * 512 : (i + 1) * 512],
                            start=(ktile == 0),
                            stop=(ktile == nkt - 1),
                        )

            def emit_normalize(p):
                qw_, pp_ = phases[p]
                av_ = av_by_phase[p]
                for h2 in range(2):
                    # denominator row must be copied to a partition-0 SBUF
                    # tile first: the custom-DVE reciprocal cannot read at
                    # partition offset 64 (baseline-proven chain)
                    rt = r_pool.tile([1, 512], f32, tag="rt", name="rt")
                    nc.vector.tensor_copy(rt[0:1, :], av_[h2][DH : DH + 1, :])
                    ri = r_pool.tile([1, 512], f32, tag="ri", name="ri")
                    nc.vector.reciprocal_approx_fast(ri[0:1, :], rt[0:1, :])
                    rb = rb_pool.tile([64, 512], f32, tag="rb", name="rb")
                    nc.gpsimd.partition_broadcast(rb[:], ri[0:1, :])
                    nc.vector.tensor_mul(
                        outT_sb[
                            h2 * 64 : (h2 + 1) * 64,
                            pp_,
                            qw_ * 512 : (qw_ + 1) * 512,
                        ],
                        av_[h2][0:DH, :],
                        rb[:],
                    )

            # background units per phase (hard deps enforced by Tile; this
            # ordering shapes engine-queue pacing and respects DMA arrival).
            # Phase p's K'/Q' inputs must be emitted during phase p-1 at the
            # latest so the strict-FIFO PE queue never stalls on them.
            # phase 0: K' pieces for pp0 must be emitted before the scores
            # chunks that read them (program order IS the data, Tile does
            # not reorder a read ahead of a later write); V' chains before
            # the lagged attnV drains; pp1's K'/Q' before phase 1.
            ph0 = [("q", 1, 0)]
            if len(kp) > 1:
                ph0.append(("k", 0, 1))
            ph0 += [("v", 0), ("v", 1)]
            if len(kp) > 2:
                ph0.append(("k", 0, 2))
            ph0.append(("k", 1, 0))
            ph0.append(("v", 2))
            if len(kp) > 1:
                ph0.append(("k", 1, 1))
            ph0.append(("v", 3))
            if len(kp) > 2:
                ph0.append(("k", 1, 2))
            ph0 += [("v", i) for i in range(4, nkt)]
            ph1 = [("q", 0, 1)]
            bg_by_phase = {
                0: ph0,
                1: ph1,
                2: [("q", 1, 1), ("wo", 0), ("wo", 1)],
                3: [("q", 0, 2), ("wo", 2), ("wo", 3)],
                4: [("q", 1, 2), ("wo", 4), ("wo", 5)],
                5: [("q", 0, 3), ("wo", 6), ("wo", 7)],
                6: [("q", 1, 3), ("wo", 8), ("wo", 9)],
                7: [("wo", 10), ("wo", 11)],
            }

            # ---- prologue: minimum inputs for phase 0's first groups -------
            emit_k_piece(0, 0, pool=psA)
            emit_q_piece(0, 0, pool=psB)

            # ---- main phase loop -------------------------------------------
            for p in range(len(phases)):
                units = list(bg_by_phase.get(p, []))
                for gi in range(len(groups)):
                    emit_scores_group(p, gi)
                    if p > 0 and gi == 1:
                        # finish previous phase: attnV remainder + normalize
                        drain_attnv(p - 1, 0)
                        emit_normalize(p - 1)
                    if units:
                        run_unit(units.pop(0))
                    if p == len(phases) - 1 and gi >= 3:
                        drain_attnv(p, 1, max_n=2)
                    else:
                        drain_attnv(p, 2, max_n=2)
                for u in units:
                    run_unit(u)

            # ---- tail: finish last phase, normalize, last Wo group ---------
            last = len(phases) - 1
            drain_attnv(last, 0)
            emit_normalize(last)
            emit_wo_tt(12, pool=psA)
            emit_wo_tt(13, pool=psB)
            emit_wo_tt(14)
            emit_wo_tt(15, pool=psA)

    nc.compile()
    return nc


_NC_CACHE: dict[int, object] = {}


def _get_nc(nkt=NKT_DEFAULT):
    if nkt not in _NC_CACHE:
        _NC_CACHE[nkt] = build_nc(nkt)
    return _NC_CACHE[nkt]


def make_in_maps(x, mask, Wq, Wk, Wv, Wo, nkt=None):
    bf = ml_dtypes.bfloat16
    mask = np.asarray(mask)
    counts = (mask > 0).sum(axis=1)
    if nkt is None:
        nkt = max(1, int(math.ceil(counts.max() / 128)))
    SK = nkt * 128

    xqT = np.ascontiguousarray(x.transpose(0, 2, 1)).astype(bf)  # [B, D, S]
    # [H, D, DH] -> [D, H*DH]
    wq_f = np.ascontiguousarray(Wq.transpose(1, 0, 2).reshape(D, H * DH))
    wk_f = np.ascontiguousarray(Wk.transpose(1, 0, 2).reshape(D, H * DH))
    wv_f = np.ascontiguousarray(Wv.transpose(1, 0, 2).reshape(D, H * DH))

    xkT = []
    vmasks = []
    for b in range(B):
        idx = np.flatnonzero(mask[b] > 0)
        nv = len(idx)
        xk_b = np.zeros((SK, D), np.float32)
        xk_b[:nv] = x[b][idx]
        xkT.append(np.ascontiguousarray(xk_b.T).astype(bf))
        vm = np.zeros((128, nkt, HPC), np.float32)
        slot = np.arange(nkt * 128).reshape(nkt, 128)
        vm[:, :, :] = (slot.T[:, :, None] < nv).astype(np.float32)
        vmasks.append(vm.reshape(128, nkt * HPC).astype(bf))

    in_maps = []
    for c in range(N_CORES):
        b, hg = c // 2, c % 2
        cols = slice(hg * HE, (hg + 1) * HE)
        in_maps.append(
            {
                "xq": xqT[b],
                "xk": xkT[b],
                "wq": np.ascontiguousarray(wq_f[:, cols]).astype(bf),
                "wk": np.ascontiguousarray(wk_f[:, cols]).astype(bf),
                "wv": np.ascontiguousarray(wv_f[:, cols]).astype(bf),
                "wo": np.ascontiguousarray(Wo[cols, :]).astype(bf),
                "vmask": vmasks[b],
            }
        )
    return in_maps, nkt


def combine_results(results):
    y = np.zeros((B, S, D), np.float32)
    for c in range(N_CORES):
        y[c // 2] += results[c]["y"].astype(np.float32)
    return y


def kernel(x, mask, Wq, Wk, Wv, Wo):
    in_maps, nkt = make_in_maps(
        np.asarray(x, np.float32),
        np.asarray(mask),
        np.asarray(Wq, np.float32),
        np.asarray(Wk, np.float32),
        np.asarray(Wv, np.float32),
        np.asarray(Wo, np.float32),
    )
    nc = _get_nc(nkt)
    res = run_bass_kernel_spmd(nc, in_maps, core_ids=list(range(N_CORES)))
    return combine_results(res.results)


# revision 19
# speedup vs baseline: 1.2853x; 1.0256x over previous
"""Trainium2 Bass kernel for nn_Encoder (B=4, S=2048, D=512, H=8 self-attention).

Sharding over 8 NeuronCores: core c -> (batch b = c//2, head-group hg = c%2).
Each core computes, for its batch and its 4 heads, the full attention block
plus a partial output projection y_part = attn_out @ Wo[group rows]. The host
sums the two partial y tensors per batch.

v3: host-side key compaction. The key-padding mask kills ~half the keys
(exp(-1e9) == 0), so the host gathers only the valid key columns of x and
pads to SK = NKT*128 slots (NKT=9 for the ~1044-valid-key regime). Pad
slots are zero columns: their scores are exactly 0, so exp gives exactly
1.0, and they are excluded exactly by (a) V' pad rows being zero (numerator)
and (b) the denominator "ones column" of V' holding the validity mask
instead of all-ones. No attention bias input is needed at all.

The Scalar engine Exp stream is the roofline (~75us for 8 phases). Scores
for (key-tile, head) chunks land in two ping-pong PSUM pools (3 banks +
2 banks) so each ACTIVATE covers N=1536/N=1024 elements, amortizing the
~220-cycle instruction overhead while leaving 3 PSUM banks for the attnV
accumulators ([65, 1024]: dh rows + denominator row, 2 heads) and one for
projection scratch. Score matmuls for the two heads of a pair alternate
row groups 0-63/64-127 so they run concurrently on the PE sub-arrays.
"""

import math

import ml_dtypes
import numpy as np

import concourse.mybir as mybir
import concourse.tile as tile
from concourse import bacc
from concourse.bass_utils import run_bass_kernel_spmd

B, S, D, H = 4, 2048, 512, 8
DH = D // H          # 64
HPC = H // 2         # 4 heads per core
HE = HPC * DH        # 256 output-proj rows per core
T = S
NDC = D // 128       # 4 contraction chunks for projections
NTW = T // 512       # 4 query windows
N_CORES = 8
NKT_DEFAULT = 9      # key tiles (128 keys each) after compaction

f32 = mybir.dt.float32
bf16 = mybir.dt.bfloat16
EXP = mybir.ActivationFunctionType.Exp


def _group_sizes(n_chunks):
    """Split n_chunks score chunks into ACT groups alternating 3/2 wide."""
    sizes = []
    want = 3
    rem = n_chunks
    while rem > 0:
        take = min(want, rem)
        sizes.append(take)
        rem -= take
        want = 2 if want == 3 else 3
    return sizes


def build_nc(nkt=NKT_DEFAULT):
    SK = nkt * 128
    nc = bacc.Bacc("TRN2", target_bir_lowering=False, debug=False, num_devices=1)

    xq = nc.dram_tensor("xq", [D, S], bf16, kind="ExternalInput").ap()
    xk = nc.dram_tensor("xk", [D, SK], bf16, kind="ExternalInput").ap()
    wq = nc.dram_tensor("wq", [D, HE], bf16, kind="ExternalInput").ap()
    wk = nc.dram_tensor("wk", [D, HE], bf16, kind="ExternalInput").ap()
    wv = nc.dram_tensor("wv", [D, HE], bf16, kind="ExternalInput").ap()
    wo = nc.dram_tensor("wo", [HE, D], bf16, kind="ExternalInput").ap()
    vmask = nc.dram_tensor("vmask", [128, nkt * HPC], bf16, kind="ExternalInput").ap()
    y = nc.dram_tensor("y", [T, D], bf16, kind="ExternalOutput").ap()

    # K-piece boundaries for the K' projection (rhs free <= 512)
    kp = []
    off = 0
    while off < SK:
        w = min(512, SK - off)
        kp.append((off, w))
        off += w

    n_chunks = 2 * nkt
    gsizes = _group_sizes(n_chunks)

    with tile.TileContext(nc) as tc:
        with (
            tc.tile_pool(name="const", bufs=1) as const,
            tc.tile_pool(name="psA", bufs=1, space="PSUM") as psA,
            tc.tile_pool(name="psB", bufs=1, space="PSUM") as psB,
            tc.tile_pool(name="psAV", bufs=2, space="PSUM") as psAV,
            tc.tile_pool(name="psM", bufs=1, space="PSUM") as psM,
            tc.tile_pool(name="atp", bufs=8) as at_pool,
            tc.tile_pool(name="yout", bufs=3) as y_pool,
            tc.tile_pool(name="recip", bufs=4) as r_pool,
            tc.tile_pool(name="recipb", bufs=4) as rb_pool,
        ):
            # ---- ACT table warm-up + PE warm-up source tiles ---------------
            warm_i = const.tile([1, 1], f32, tag="warm_i")
            nc.gpsimd.memset(warm_i[:], 0.0)
            ones33 = const.tile([33, 64], bf16, tag="ones33")
            nc.gpsimd.memset(ones33[:], 1.0)
            warm_o = const.tile([1, 1], f32, tag="warm_o")
            nc.scalar.activation(warm_o[:], warm_i[:], EXP)

            # ---- loads (priority order: first-needed first) ----------------
            wk_sb = const.tile([128, NDC, HE], bf16, tag="wk")
            nc.scalar.dma_start(wk_sb[:], wk.rearrange("(c p) n -> p c n", p=128))
            xk_sb = const.tile([128, NDC, SK], bf16, tag="xk")
            xk_r = xk.rearrange("(c p) s -> c p s", p=128)

            xk_r2 = xk.rearrange("(c p) s -> p c s", p=128)

            def dma_xk_piece(off, w):
                nc.sync.dma_start(
                    xk_sb[:, :, off : off + w], xk_r2[:, :, off : off + w]
                )

            dma_xk_piece(0, 512)
            wq_sb = const.tile([128, NDC, HE], bf16, tag="wq")
            nc.scalar.dma_start(wq_sb[:], wq.rearrange("(c p) n -> p c n", p=128))
            xq_sb = const.tile([128, NDC, S], bf16, tag="xq")
            xq_r = xq.rearrange("(c p) s -> c p s", p=128)

            xq_r2 = xq.rearrange("(c p) s -> p c s", p=128)

            def dma_xq_piece(qw, eng=None):
                (eng or nc.sync).dma_start(
                    xq_sb[:, :, qw * 512 : (qw + 1) * 512],
                    xq_r2[:, :, qw * 512 : (qw + 1) * 512],
                )

            dma_xq_piece(0, eng=nc.gpsimd)
            dma_xk_piece(512, min(512, SK - 512))
            wv_sb = const.tile([128, NDC, HE], bf16, tag="wv")
            nc.gpsimd.dma_start(wv_sb[:], wv.rearrange("(c p) n -> p c n", p=128))
            if SK > 1024:
                dma_xk_piece(1024, SK - 1024)

            # V' tiles: [partition(key in tile)][key-tile][local head][DH + vcol]
            # column DH holds the validity mask (1 valid / 0 pad) so the
            # denominator row of attnV excludes pad keys exactly.
            v_sb = const.tile([128, nkt, HPC, DH + 1], bf16, tag="v")
            nc.sync.dma_start(
                v_sb[:, :, :, DH],
                vmask.rearrange("p (j h) -> p j h", h=HPC),
            )
            dma_xq_piece(1)
            dma_xq_piece(2)
            dma_xq_piece(3)
            wo_sb = const.tile([128, HE // 128, D], bf16, tag="wo")
            nc.sync.dma_start(wo_sb[:], wo.rearrange("(c p) n -> p c n", p=128))

            kt = [
                const.tile([128, SK], bf16, tag=f"kt{pp}", name=f"kt{pp}")
                for pp in range(2)
            ]
            qt = [
                const.tile([128, S], bf16, tag=f"qt{pp}", name=f"qt{pp}")
                for pp in range(2)
            ]
            # outT [he, t] as [128, 2, T]: chunk pp, rows h2*64
            outT_sb = const.tile([128, HE // 128, T], bf16, tag="outT")

            # PE keep-warm: tiny matmuls with no data deps run during the
            # DMA window so the HAM clock gate stays open.
            def emit_pe_warm(n):
                warm_ps = psAV.tile([64, 64], f32, tag="av", name="warm_ps")
                for _ in range(n):
                    nc.tensor.matmul(
                        warm_ps[:],
                        lhsT=ones33[0:1, :],
                        rhs=ones33[0:1, :],
                        start=True,
                        stop=True,
                    )

            emit_pe_warm(70)

            # ---- background work units -------------------------------------
            kq_done: set[tuple] = set()
            v_done = [False] * nkt

            def emit_k_piece(pp, pi, pool=None):
                key = ("k", pp, pi)
                if key in kq_done:
                    return
                kq_done.add(key)
                off, w = kp[pi]
                ps = (pool or psM).tile(
                    [128, 512], f32, tag="mm" if pool is None else "sc", name="k_ps"
                )
                for dc in range(NDC):
                    nc.tensor.matmul(
                        ps[:, 0:w],
                        lhsT=wk_sb[:, dc, pp * 128 : (pp + 1) * 128],
                        rhs=xk_sb[:, dc, off : off + w],
                        start=(dc == 0),
                        stop=(dc == NDC - 1),
                    )
                nc.vector.tensor_copy(kt[pp][:, off : off + w], ps[:, 0:w])

            def emit_q_piece(pp, qw, pool=None):
                key = ("q", pp, qw)
                if key in kq_done:
                    return
                kq_done.add(key)
                ps = (pool or psM).tile(
                    [128, 512], f32, tag="mm" if pool is None else "sc", name="q_ps"
                )
                for dc in range(NDC):
                    nc.tensor.matmul(
                        ps[:],
                        lhsT=wq_sb[:, dc, pp * 128 : (pp + 1) * 128],
                        rhs=xq_sb[:, dc, qw * 512 : (qw + 1) * 512],
                        start=(dc == 0),
                        stop=(dc == NDC - 1),
                    )
                nc.vector.tensor_copy(qt[pp][:, qw * 512 : (qw + 1) * 512], ps[:])

            def emit_v_chain(vst, pool=None):
                if v_done[vst]:
                    return
                v_done[vst] = True
                ps = (pool or psM).tile(
                    [128, HE], f32, tag="mm" if pool is None else "sc", name="v_ps"
                )
                for dc in range(NDC):
                    nc.tensor.matmul(
                        ps[:],
                        lhsT=xk_sb[:, dc, vst * 128 : (vst + 1) * 128],
                        rhs=wv_sb[:, dc, :],
                        start=(dc == 0),
                        stop=(dc == NDC - 1),
                    )
                nc.vector.tensor_copy(
                    v_sb[:, vst, :, 0:DH],
                    ps[:].rearrange("p (h e) -> p h e", e=DH),
                )

            def emit_wo_tt(tt, pool=None):
                ps = (pool or psM).tile(
                    [128, 512], f32, tag="mm" if pool is None else "sc", name="y_ps"
                )
                for c in range(HE // 128):
                    nc.tensor.matmul(
                        ps[:],
                        lhsT=outT_sb[:, c, tt * 128 : (tt + 1) * 128],
                        rhs=wo_sb[:, c, :],
                        start=(c == 0),
                        stop=(c == HE // 128 - 1),
                    )
                y_sb = y_pool.tile([128, 512], bf16, tag="y", name="y_sb")
                nc.vector.tensor_copy(y_sb[:], ps[:])
                nc.gpsimd.dma_start(y[tt * 128 : (tt + 1) * 128, :], y_sb[:])

            def run_unit(u):
                if u[0] == "v":
                    emit_v_chain(u[1])
                elif u[0] == "k":
                    emit_k_piece(u[1], u[2])
                elif u[0] == "q":
                    emit_q_piece(u[1], u[2])
                else:
                    emit_wo_tt(u[1])

            # phases: qw-major, pp-inner so Wo(qw) unblocks early
            phases = [(qw, pp) for qw in range(NTW) for pp in range(2)]

            # chunk c of a phase -> (key tile, head-in-pair)
            # groups partition the 2*nkt chunks into ACT-sized pieces
            groups = []
            c0 = 0
            for gs in gsizes:
                groups.append(list(range(c0, c0 + gs)))
                c0 += gs

            pend: list[tuple] = []   # (phase_idx, at_tile, chunk_list)
            av_by_phase: dict[int, object] = {}

            def emit_scores_group(p, gi):
                qw, pp = phases[p]
                chunks = groups[gi]
                pool = psA if gsizes[gi] == 3 else psB
                width = gsizes[gi] * 512
                sc = pool.tile([128, width], f32, tag="sc", name="sc")
                with tc.high_priority(offset=40):
                    for i, c in enumerate(chunks):
                        ktile, h2 = c // 2, c % 2
                        nc.tensor.matmul(
                            sc[:, i * 512 : (i + 1) * 512],
                            lhsT=kt[pp][
                                h2 * 64 : (h2 + 1) * 64,
                                ktile * 128 : (ktile + 1) * 128,
                            ],
                            rhs=qt[pp][
                                h2 * 64 : (h2 + 1) * 64, qw * 512 : (qw + 1) * 512
                            ],
                            start=True,
                            stop=True,
                        )
                at_t = at_pool.tile([128, width], bf16, tag="at", name="at")
                nc.scalar.activation(at_t[:], sc[:], EXP, scale=0.125)
                pend.append((p, at_t, chunks))

            def drain_attnv(upto_p, min_keep, max_n=1000):
                n = 0
                while pend and len(pend) > min_keep and pend[0][0] <= upto_p and n < max_n:
                    n += 1
                    p_, at_t, chunks = pend.pop(0)
                    qw_, pp_ = phases[p_]
                    for c in chunks:
                        if not v_done[c // 2]:
                            emit_v_chain(c // 2)
                    if p_ not in av_by_phase:
                        av_by_phase[p_] = [
                            psAV.tile([DH + 1, 512], f32, tag="av", name=f"av{h2}")
                            for h2 in range(2)
                        ]
                    av_ = av_by_phase[p_]
                    for i, c in enumerate(chunks):
                        ktile, h2 = c // 2, c % 2
                        nc.tensor.matmul(
                            av_[h2][:],
                            lhsT=v_sb[:, ktile, 2 * pp_ + h2, :],
                            rhs=at_t[:, i * 512 : (i + 1) * 512],
                            start=(ktile == 0),
                            stop=(ktile == nkt - 1),
                        )

            def emit_normalize(p):
                qw_, pp_ = phases[p]
                av_ = av_by_phase[p]
                for h2 in range(2):
                    # denominator row must be copied to a partition-0 SBUF
                    # tile first: the custom-DVE reciprocal cannot read at
                    # partition offset 64 (baseline-proven chain)
                    rt = r_pool.tile([1, 512], f32, tag="rt", name="rt")
                    nc.vector.tensor_copy(rt[0:1, :], av_[h2][DH : DH + 1, :])
                    ri = r_pool.tile([1, 512], f32, tag="ri", name="ri")
                    nc.vector.reciprocal_approx_fast(ri[0:1, :], rt[0:1, :])
                    rb = rb_pool.tile([64, 512], f32, tag="rb", name="rb")
                    nc.gpsimd.partition_broadcast(rb[:], ri[0:1, :])
                    nc.vector.tensor_mul(
                        outT_sb[
                            h2 * 64 : (h2 + 1) * 64,
                            pp_,
                            qw_ * 512 : (qw_ + 1) * 512,
                        ],
                        av_[h2][0:DH, :],
                        rb[:],
                    )

            # background units per phase (hard deps enforced by Tile; this
            # ordering shapes engine-queue pacing and respects DMA arrival).
            # Phase p's K'/Q' inputs must be emitted during phase p-1 at the
            # latest so the strict-FIFO PE queue never stalls on them.
            # phase 0: K' pieces for pp0 must be emitted before the scores
            # chunks that read them (program order IS the data, Tile does
            # not reorder a read ahead of a later write); V' chains before
            # the lagged attnV drains; pp1's K'/Q' before phase 1.
            ph0 = [("q", 1, 0)]
            if len(kp) > 1:
                ph0.append(("k", 0, 1))
            ph0 += [("v", 0), ("v", 1)]
            if len(kp) > 2:
                ph0.append(("k", 0, 2))
            ph0.append(("k", 1, 0))
            ph0.append(("v", 2))
            if len(kp) > 1:
                ph0.append(("k", 1, 1))
            ph0.append(("v", 3))
            if len(kp) > 2:
                ph0.append(("k", 1, 2))
            ph0 += [("v", i) for i in range(4, nkt)]
            ph1 = [("q", 0, 1)]
            bg_by_phase = {
                0: ph0,
                1: ph1,
                2: [("q", 1, 1), ("wo", 0), ("wo", 1)],
                3: [("q", 0, 2), ("wo", 2), ("wo", 3)],
                4: [("q", 1, 2), ("wo", 4), ("wo", 5)],
                5: [("q", 0, 3), ("wo", 6), ("wo", 7)],
                6: [("q", 1, 3), ("wo", 8), ("wo", 9)],
                7: [("wo", 10), ("wo", 11)],
            }

            # ---- prologue: minimum inputs for phase 0's first groups -------
            emit_k_piece(0, 0, pool=psA)
            emit_q_piece(0, 0, pool=psB)

            # ---- main phase loop -------------------------------------------
            for p in range(len(phases)):
                units = list(bg_by_phase.get(p, []))
                for gi in range(len(groups)):
                    emit_scores_group(p, gi)
                    if p > 0 and gi == 0:
                        drain_attnv(p - 1, 0, max_n=2)
                    if p > 0 and gi == 1:
                        # finish previous phase: attnV remainder + normalize
                        drain_attnv(p - 1, 0)
                        emit_normalize(p - 1)
                    if units:
                        run_unit(units.pop(0))
                    if p == len(phases) - 1 and gi >= 2:
                        drain_attnv(p, 1, max_n=2)
                    else:
                        drain_attnv(p, 2, max_n=1)
                for u in units:
                    run_unit(u)

            # ---- tail: finish last phase, normalize, last Wo group ---------
            last = len(phases) - 1
            drain_attnv(last, 0)
            emit_normalize(last)
            emit_wo_tt(12, pool=psA)
            emit_wo_tt(13, pool=psB)
            emit_wo_tt(14)
            emit_wo_tt(15, pool=psA)

    nc.compile()
    return nc


_NC_CACHE: dict[int, object] = {}


def _get_nc(nkt=NKT_DEFAULT):
    if nkt not in _NC_CACHE:
        _NC_CACHE[nkt] = build_nc(nkt)
    return _NC_CACHE[nkt]


def make_in_maps(x, mask, Wq, Wk, Wv, Wo, nkt=None):
    bf = ml_dtypes.bfloat16
    mask = np.asarray(mask)
    counts = (mask > 0).sum(axis=1)
    if nkt is None:
        nkt = max(1, int(math.ceil(counts.max() / 128)))
    SK = nkt * 128

    xqT = np.ascontiguousarray(x.transpose(0, 2, 1)).astype(bf)  # [B, D, S]
    # [H, D, DH] -> [D, H*DH]
    wq_f = np.ascontiguousarray(Wq.transpose(1, 0, 2).reshape(D, H * DH))
    wk_f = np.ascontiguousarray(Wk.transpose(1, 0, 2).reshape(D, H * DH))
    wv_f = np.ascontiguousarray(Wv.transpose(1, 0, 2).reshape(D, H * DH))

    xkT = []
    vmasks = []
    for b in range(B):
        idx = np.flatnonzero(mask[b] > 0)
        nv = len(idx)
        xk_b = np.zeros((SK, D), np.float32)
        xk_b[:nv] = x[b][idx]
        xkT.append(np.ascontiguousarray(xk_b.T).astype(bf))
        vm = np.zeros((128, nkt, HPC), np.float32)
        slot = np.arange(nkt * 128).reshape(nkt, 128)
        vm[:, :, :] = (slot.T[:, :, None] < nv).astype(np.float32)
        vmasks.append(vm.reshape(128, nkt * HPC).astype(bf))

    in_maps = []
    for c in range(N_CORES):
        b, hg = c // 2, c % 2
        cols = slice(hg * HE, (hg + 1) * HE)
        in_maps.append(
            {
                "xq": xqT[b],
                "xk": xkT[b],
                "wq": np.ascontiguousarray(wq_f[:, cols]).astype(bf),
                "wk": np.ascontiguousarray(wk_f[:, cols]).astype(bf),
                "wv": np.ascontiguousarray(wv_f[:, cols]).astype(bf),
                "wo": np.ascontiguousarray(Wo[cols, :]).astype(bf),
                "vmask": vmasks[b],
            }
        )
    return in_maps, nkt


def combine_results(results):
    y = np.zeros((B, S, D), np.float32)
    for c in range(N_CORES):
        y[c // 2] += results[c]["y"].astype(np.float32)
    return y


def kernel(x, mask, Wq, Wk, Wv, Wo):
    in_maps, nkt = make_in_maps(
        np.asarray(x, np.float32),
        np.asarray(mask),
        np.asarray(Wq, np.float32),
        np.asarray(Wk, np.float32),
        np.asarray(Wv, np.float32),
        np.asarray(Wo, np.float32),
    )
    nc = _get_nc(nkt)
    res = run_bass_kernel_spmd(nc, in_maps, core_ids=list(range(N_CORES)))
    return combine_results(res.results)


# revision 21
# speedup vs baseline: 1.3332x; 1.0372x over previous
"""Trainium2 Bass kernel for nn_Encoder (B=4, S=2048, D=512, H=8 self-attention).

Sharding over 8 NeuronCores: core c -> (batch b = c//2, head-group hg = c%2).
Each core computes, for its batch and its 4 heads, the full attention block
plus a partial output projection y_part = attn_out @ Wo[group rows]. The host
sums the two partial y tensors per batch.

v3: host-side key compaction. The key-padding mask kills ~half the keys
(exp(-1e9) == 0), so the host gathers only the valid key columns of x and
pads to SK = NKT*128 slots (NKT=9 for the ~1044-valid-key regime). Pad
slots are zero columns: their scores are exactly 0, so exp gives exactly
1.0, and they are excluded exactly by (a) V' pad rows being zero (numerator)
and (b) the denominator "ones column" of V' holding the validity mask
instead of all-ones. No attention bias input is needed at all.

The Scalar engine Exp stream is the roofline (~75us for 8 phases). Scores
for (key-tile, head) chunks land in two ping-pong PSUM pools (3 banks +
2 banks) so each ACTIVATE covers N=1536/N=1024 elements, amortizing the
~220-cycle instruction overhead while leaving 3 PSUM banks for the attnV
accumulators ([65, 1024]: dh rows + denominator row, 2 heads) and one for
projection scratch. Score matmuls for the two heads of a pair alternate
row groups 0-63/64-127 so they run concurrently on the PE sub-arrays.
"""

import math

import ml_dtypes
import numpy as np

import concourse.mybir as mybir
import concourse.tile as tile
from concourse import bacc
from concourse.bass_utils import run_bass_kernel_spmd

B, S, D, H = 4, 2048, 512, 8
DH = D // H          # 64
HPC = H // 2         # 4 heads per core
HE = HPC * DH        # 256 output-proj rows per core
T = S
NDC = D // 128       # 4 contraction chunks for projections
NTW = T // 512       # 4 query windows
N_CORES = 8
NKT_DEFAULT = 9      # key tiles (128 keys each) after compaction

f32 = mybir.dt.float32
bf16 = mybir.dt.bfloat16
EXP = mybir.ActivationFunctionType.Exp


def _group_sizes(n_chunks):
    """Split n_chunks score chunks into ACT groups alternating 3/2 wide."""
    sizes = []
    want = 3
    rem = n_chunks
    while rem > 0:
        take = min(want, rem)
        sizes.append(take)
        rem -= take
        want = 2 if want == 3 else 3
    return sizes


def build_nc(nkt=NKT_DEFAULT):
    SK = nkt * 128
    nc = bacc.Bacc("TRN2", target_bir_lowering=False, debug=False, num_devices=1)

    xq = nc.dram_tensor("xq", [D, S], bf16, kind="ExternalInput").ap()
    xk = nc.dram_tensor("xk", [D, SK], bf16, kind="ExternalInput").ap()
    wq = nc.dram_tensor("wq", [D, HE], bf16, kind="ExternalInput").ap()
    wk = nc.dram_tensor("wk", [D, HE], bf16, kind="ExternalInput").ap()
    wv = nc.dram_tensor("wv", [D, HE], bf16, kind="ExternalInput").ap()
    wo = nc.dram_tensor("wo", [HE, D], bf16, kind="ExternalInput").ap()
    vmask = nc.dram_tensor("vmask", [128, nkt * HPC], bf16, kind="ExternalInput").ap()
    y = nc.dram_tensor("y", [T, D], bf16, kind="ExternalOutput").ap()

    # K-piece boundaries for the K' projection (rhs free <= 512)
    kp = []
    off = 0
    while off < SK:
        w = min(512, SK - off)
        kp.append((off, w))
        off += w

    n_chunks = 2 * nkt
    gsizes = _group_sizes(n_chunks)

    with tile.TileContext(nc) as tc:
        with (
            tc.tile_pool(name="const", bufs=1) as const,
            tc.tile_pool(name="psA", bufs=1, space="PSUM") as psA,
            tc.tile_pool(name="psB", bufs=1, space="PSUM") as psB,
            tc.tile_pool(name="psAV", bufs=2, space="PSUM") as psAV,
            tc.tile_pool(name="psM", bufs=1, space="PSUM") as psM,
            tc.tile_pool(name="atp", bufs=8) as at_pool,
            tc.tile_pool(name="yout", bufs=3) as y_pool,
            tc.tile_pool(name="recip", bufs=4) as r_pool,
            tc.tile_pool(name="recipb", bufs=4) as rb_pool,
        ):
            # ---- ACT table warm-up + PE warm-up source tiles ---------------
            warm_i = const.tile([1, 1], f32, tag="warm_i")
            nc.gpsimd.memset(warm_i[:], 0.0)
            ones33 = const.tile([33, 64], bf16, tag="ones33")
            nc.gpsimd.memset(ones33[:], 1.0)
            warm_o = const.tile([1, 1], f32, tag="warm_o")
            nc.scalar.activation(warm_o[:], warm_i[:], EXP)

            # ---- loads (priority order: first-needed first) ----------------
            wk_sb = const.tile([128, NDC, HE], bf16, tag="wk")
            nc.scalar.dma_start(wk_sb[:], wk.rearrange("(c p) n -> p c n", p=128))
            xk_sb = const.tile([128, NDC, SK], bf16, tag="xk")
            xk_r = xk.rearrange("(c p) s -> c p s", p=128)

            xk_r2 = xk.rearrange("(c p) s -> p c s", p=128)

            def dma_xk_piece(off, w):
                nc.sync.dma_start(
                    xk_sb[:, :, off : off + w], xk_r2[:, :, off : off + w]
                )

            dma_xk_piece(0, 512)
            wq_sb = const.tile([128, NDC, HE], bf16, tag="wq")
            nc.scalar.dma_start(wq_sb[:], wq.rearrange("(c p) n -> p c n", p=128))
            xq_sb = const.tile([128, NDC, S], bf16, tag="xq")
            xq_r = xq.rearrange("(c p) s -> c p s", p=128)

            xq_r2 = xq.rearrange("(c p) s -> p c s", p=128)

            def dma_xq_piece(qw, eng=None):
                (eng or nc.sync).dma_start(
                    xq_sb[:, :, qw * 512 : (qw + 1) * 512],
                    xq_r2[:, :, qw * 512 : (qw + 1) * 512],
                )

            dma_xq_piece(0, eng=nc.gpsimd)
            dma_xk_piece(512, min(512, SK - 512))
            wv_sb = const.tile([128, NDC, HE], bf16, tag="wv")
            nc.gpsimd.dma_start(wv_sb[:], wv.rearrange("(c p) n -> p c n", p=128))
            if SK > 1024:
                dma_xk_piece(1024, SK - 1024)

            # V' tiles: [partition(key in tile)][key-tile][local head][DH + vcol]
            # column DH holds the validity mask (1 valid / 0 pad) so the
            # denominator row of attnV excludes pad keys exactly.
            v_sb = const.tile([128, nkt, HPC, DH + 1], bf16, tag="v")
            vm_sb = const.tile([128, nkt * HPC], bf16, tag="vm")
            nc.sync.dma_start(vm_sb[:], vmask[:, :])
            nc.vector.tensor_copy(
                v_sb[:, :, :, DH],
                vm_sb[:].rearrange("p (j h) -> p j h", h=HPC),
            )
            dma_xq_piece(1)
            dma_xq_piece(2)
            dma_xq_piece(3)
            wo_sb = const.tile([128, HE // 128, D], bf16, tag="wo")
            nc.sync.dma_start(wo_sb[:], wo.rearrange("(c p) n -> p c n", p=128))

            kt = [
                const.tile([128, SK], bf16, tag=f"kt{pp}", name=f"kt{pp}")
                for pp in range(2)
            ]
            qt = [
                const.tile([128, S], bf16, tag=f"qt{pp}", name=f"qt{pp}")
                for pp in range(2)
            ]
            # outT [he, t] as [128, 2, T]: chunk pp, rows h2*64
            outT_sb = const.tile([128, HE // 128, T], bf16, tag="outT")

            # PE keep-warm: tiny matmuls with no data deps run during the
            # DMA window so the HAM clock gate stays open.
            def emit_pe_warm(n):
                warm_ps = psAV.tile([64, 64], f32, tag="av", name="warm_ps")
                for _ in range(n):
                    nc.tensor.matmul(
                        warm_ps[:],
                        lhsT=ones33[0:1, :],
                        rhs=ones33[0:1, :],
                        start=True,
                        stop=True,
                    )

            emit_pe_warm(70)

            # ---- background work units -------------------------------------
            kq_done: set[tuple] = set()
            v_done = [False] * nkt

            def emit_k_piece(pp, pi, pool=None):
                key = ("k", pp, pi)
                if key in kq_done:
                    return
                kq_done.add(key)
                off, w = kp[pi]
                ps = (pool or psM).tile(
                    [128, 512], f32, tag="mm" if pool is None else "sc", name="k_ps"
                )
                for dc in range(NDC):
                    nc.tensor.matmul(
                        ps[:, 0:w],
                        lhsT=wk_sb[:, dc, pp * 128 : (pp + 1) * 128],
                        rhs=xk_sb[:, dc, off : off + w],
                        start=(dc == 0),
                        stop=(dc == NDC - 1),
                    )
                nc.vector.tensor_copy(kt[pp][:, off : off + w], ps[:, 0:w])

            def emit_q_piece(pp, qw, pool=None):
                key = ("q", pp, qw)
                if key in kq_done:
                    return
                kq_done.add(key)
                ps = (pool or psM).tile(
                    [128, 512], f32, tag="mm" if pool is None else "sc", name="q_ps"
                )
                for dc in range(NDC):
                    nc.tensor.matmul(
                        ps[:],
                        lhsT=wq_sb[:, dc, pp * 128 : (pp + 1) * 128],
                        rhs=xq_sb[:, dc, qw * 512 : (qw + 1) * 512],
                        start=(dc == 0),
                        stop=(dc == NDC - 1),
                    )
                nc.vector.tensor_copy(qt[pp][:, qw * 512 : (qw + 1) * 512], ps[:])

            def emit_v_chain(vst, pool=None):
                if v_done[vst]:
                    return
                v_done[vst] = True
                ps = (pool or psM).tile(
                    [128, HE], f32, tag="mm" if pool is None else "sc", name="v_ps"
                )
                for dc in range(NDC):
                    nc.tensor.matmul(
                        ps[:],
                        lhsT=xk_sb[:, dc, vst * 128 : (vst + 1) * 128],
                        rhs=wv_sb[:, dc, :],
                        start=(dc == 0),
                        stop=(dc == NDC - 1),
                    )
                nc.vector.tensor_copy(
                    v_sb[:, vst, :, 0:DH],
                    ps[:].rearrange("p (h e) -> p h e", e=DH),
                )

            def emit_wo_tt(tt, pool=None, eng=None):
                ps = (pool or psM).tile(
                    [128, 512], f32, tag="mm" if pool is None else "sc", name="y_ps"
                )
                for c in range(HE // 128):
                    nc.tensor.matmul(
                        ps[:],
                        lhsT=outT_sb[:, c, tt * 128 : (tt + 1) * 128],
                        rhs=wo_sb[:, c, :],
                        start=(c == 0),
                        stop=(c == HE // 128 - 1),
                    )
                y_sb = y_pool.tile([128, 512], bf16, tag="y", name="y_sb")
                nc.vector.tensor_copy(y_sb[:], ps[:])
                (eng or nc.gpsimd).dma_start(y[tt * 128 : (tt + 1) * 128, :], y_sb[:])

            def run_unit(u):
                if u[0] == "v":
                    emit_v_chain(u[1])
                elif u[0] == "k":
                    emit_k_piece(u[1], u[2])
                elif u[0] == "q":
                    emit_q_piece(u[1], u[2])
                else:
                    # a Wo unit reads outT for its query window: both of
                    # that window's phases must have been normalized (i.e.
                    # fully drained) BEFORE this emission, else the matmul
                    # reads stale outT (program order is the data)
                    req = (u[1] // 4) * 2 + 1
                    while drained[req] < CPP:
                        if not drain_one():
                            break
                    emit_wo_tt(u[1])

            # phases: qw-major, pp-inner so Wo(qw) unblocks early
            phases = [(qw, pp) for qw in range(NTW) for pp in range(2)]
            NPH = len(phases)
            CPP = 2 * nkt          # chunks per phase
            NCH = NPH * CPP        # global chunk count

            # Global ACT-group stream: groups alternate between the 3-bank
            # (A) and 2-bank (B) PSUM pools and may SPAN phase boundaries
            # (the Exp scale is uniform), so the A/B ping-pong never stalls
            # at a phase transition.
            gsz = []
            rem, cap = NCH, 3
            while rem > 0:
                gsz.append(min(cap, rem))
                rem -= gsz[-1]
                cap = 2 if cap == 3 else 3

            def chunk_info(C):
                p, c = C // CPP, C % CPP
                return p, c // 2, c % 2   # phase, key tile, head-in-pair

            pend: list[tuple] = []       # (at_tile, [global chunk ids])
            av_by_phase: dict[int, list] = {}
            drained = [0] * NPH

            def emit_scores_group(C0, size, pool):
                width = size * 512
                sc = pool.tile([128, width], f32, tag="sc", name="sc")
                with tc.high_priority(offset=40):
                    for i in range(size):
                        p, ktile, h2 = chunk_info(C0 + i)
                        qw, pp = phases[p]
                        nc.tensor.matmul(
                            sc[:, i * 512 : (i + 1) * 512],
                            lhsT=kt[pp][
                                h2 * 64 : (h2 + 1) * 64,
                                ktile * 128 : (ktile + 1) * 128,
                            ],
                            rhs=qt[pp][
                                h2 * 64 : (h2 + 1) * 64, qw * 512 : (qw + 1) * 512
                            ],
                            start=True,
                            stop=True,
                        )
                at_t = at_pool.tile([128, width], bf16, tag="at", name="at")
                nc.scalar.activation(at_t[:], sc[:], EXP, scale=0.125)
                pend.append((at_t, list(range(C0, C0 + size))))

            def emit_normalize(p):
                qw_, pp_ = phases[p]
                av_ = av_by_phase[p]
                for h2 in range(2):
                    # denominator row must be copied to a partition-0 SBUF
                    # tile first: the custom-DVE reciprocal cannot read the
                    # PSUM row at partition offset 64 directly
                    rt = r_pool.tile([1, 512], f32, tag="rt", name="rt")
                    nc.vector.tensor_copy(rt[0:1, :], av_[h2][DH : DH + 1, :])
                    ri = r_pool.tile([1, 512], f32, tag="ri", name="ri")
                    nc.vector.reciprocal_approx_fast(ri[0:1, :], rt[0:1, :])
                    rb = rb_pool.tile([64, 512], f32, tag="rb", name="rb")
                    nc.gpsimd.partition_broadcast(rb[:], ri[0:1, :])
                    nc.vector.tensor_mul(
                        outT_sb[
                            h2 * 64 : (h2 + 1) * 64,
                            pp_,
                            qw_ * 512 : (qw_ + 1) * 512,
                        ],
                        av_[h2][0:DH, :],
                        rb[:],
                    )

            def drain_one():
                if not pend:
                    return False
                at_t, chunks = pend.pop(0)
                for C in chunks:
                    p_, ktile, h2 = chunk_info(C)
                    if not v_done[ktile]:
                        emit_v_chain(ktile)
                for i, C in enumerate(chunks):
                    p_, ktile, h2 = chunk_info(C)
                    pp_ = phases[p_][1]
                    if p_ not in av_by_phase:
                        av_by_phase[p_] = [
                            psAV.tile([DH + 1, 512], f32, tag="av", name=f"av{h}")
                            for h in range(2)
                        ]
                    nc.tensor.matmul(
                        av_by_phase[p_][h2][:],
                        lhsT=v_sb[:, ktile, 2 * pp_ + h2, :],
                        rhs=at_t[:, i * 512 : (i + 1) * 512],
                        start=(ktile == 0),
                        stop=(ktile == nkt - 1),
                    )
                    drained[p_] += 1
                    if drained[p_] == CPP:
                        emit_normalize(p_)
                return True

            # background units, scheduled per phase (hard deps enforced by
            # Tile; ordering shapes engine pacing and respects DMA arrival;
            # every K'/V'/Q' must be EMITTED before its first reader since
            # Tile does not reorder a read ahead of a later write).
            ph0 = [("q", 1, 0)]
            if len(kp) > 1:
                ph0.append(("k", 0, 1))
            ph0 += [("v", 0), ("v", 1)]
            if len(kp) > 2:
                ph0.append(("k", 0, 2))
            ph0 += [("k", 1, 0), ("v", 2)]
            if len(kp) > 1:
                ph0.append(("k", 1, 1))
            ph0.append(("v", 3))
            if len(kp) > 2:
                ph0.append(("k", 1, 2))
            ph0 += [("v", i) for i in range(4, nkt)]
            bg_by_phase = {
                0: ph0,
                1: [("q", 0, 1)],
                2: [("q", 1, 1), ("wo", 0), ("wo", 1)],
                3: [("q", 0, 2), ("wo", 2), ("wo", 3)],
                4: [("q", 1, 2), ("wo", 4), ("wo", 5)],
                5: [("q", 0, 3), ("wo", 6), ("wo", 7)],
                6: [("q", 1, 3), ("wo", 8), ("wo", 9)],
                7: [("wo", 10), ("wo", 11)],
            }

            # ---- prologue: minimum inputs for the first groups -------------
            emit_k_piece(0, 0, pool=psA)
            emit_q_piece(0, 0, pool=psB)

            # ---- main global group loop ------------------------------------
            units: list[tuple] = []
            seen_phase = -1
            C0 = 0
            for gi, size in enumerate(gsz):
                pool = psA if size == 3 else psB
                p_first = chunk_info(C0)[0]
                if p_first > seen_phase:
                    for p in range(seen_phase + 1, p_first + 1):
                        units.extend(bg_by_phase.get(p, []))
                    seen_phase = p_first
                emit_scores_group(C0, size, pool)
                C0 += size
                n_units = 2 if len(units) > 7 else 1
                for _ in range(n_units):
                    if units:
                        run_unit(units.pop(0))
                keep = 1 if gi >= len(gsz) - 8 else 2
                while len(pend) > keep:
                    if not drain_one():
                        break

            # ---- tail: drain, normalize via drained-trigger, last Wo -------
            while drain_one():
                pass
            for u in units:
                run_unit(u)
            emit_wo_tt(12, pool=psA, eng=nc.sync)
            emit_wo_tt(13, pool=psB, eng=nc.sync)
            emit_wo_tt(14, eng=nc.sync)
            emit_wo_tt(15, pool=psA, eng=nc.sync)

    nc.compile()
    return nc


_NC_CACHE: dict[int, object] = {}


def _get_nc(nkt=NKT_DEFAULT):
    if nkt not in _NC_CACHE:
        _NC_CACHE[nkt] = build_nc(nkt)
    return _NC_CACHE[nkt]


def make_in_maps(x, mask, Wq, Wk, Wv, Wo, nkt=None):
    bf = ml_dtypes.bfloat16
    mask = np.asarray(mask)
    counts = (mask > 0).sum(axis=1)
    if nkt is None:
        nkt = max(1, int(math.ceil(counts.max() / 128)))
    SK = nkt * 128

    xqT = np.ascontiguousarray(x.transpose(0, 2, 1)).astype(bf)  # [B, D, S]
    # [H, D, DH] -> [D, H*DH]
    wq_f = np.ascontiguousarray(Wq.transpose(1, 0, 2).reshape(D, H * DH))
    wk_f = np.ascontiguousarray(Wk.transpose(1, 0, 2).reshape(D, H * DH))
    wv_f = np.ascontiguousarray(Wv.transpose(1, 0, 2).reshape(D, H * DH))

    xkT = []
    vmasks = []
    for b in range(B):
        idx = np.flatnonzero(mask[b] > 0)
        nv = len(idx)
        xk_b = np.zeros((SK, D), np.float32)
        xk_b[:nv] = x[b][idx]
        xkT.append(np.ascontiguousarray(xk_b.T).astype(bf))
        vm = np.zeros((128, nkt, HPC), np.float32)
        slot = np.arange(nkt * 128).reshape(nkt, 128)
        vm[:, :, :] = (slot.T[:, :, None] < nv).astype(np.float32)
        vmasks.append(vm.reshape(128, nkt * HPC).astype(bf))

    in_maps = []
    for c in range(N_CORES):
        b, hg = c // 2, c % 2
        cols = slice(hg * HE, (hg + 1) * HE)
        in_maps.append(
            {
                "xq": xqT[b],
                "xk": xkT[b],
                "wq": np.ascontiguousarray(wq_f[:, cols]).astype(bf),
                "wk": np.ascontiguousarray(wk_f[:, cols]).astype(bf),
                "wv": np.ascontiguousarray(wv_f[:, cols]).astype(bf),
                "wo": np.ascontiguousarray(Wo[cols, :]).astype(bf),
                "vmask": vmasks[b],
            }
        )
    return in_maps, nkt


def combine_results(results):
    y = np.zeros((B, S, D), np.float32)
    for c in range(N_CORES):
        y[c // 2] += results[c]["y"].astype(np.float32)
    return y


def kernel(x, mask, Wq, Wk, Wv, Wo):
    in_maps, nkt = make_in_maps(
        np.asarray(x, np.float32),
        np.asarray(mask),
        np.asarray(Wq, np.float32),
        np.asarray(Wk, np.float32),
        np.asarray(Wv, np.float32),
        np.asarray(Wo, np.float32),
    )
    nc = _get_nc(nkt)
    res = run_bass_kernel_spmd(nc, in_maps, core_ids=list(range(N_CORES)))
    return combine_results(res.results)
